# revision 1
# baseline (speedup 1.0000x reference)
"""Self-contained Trainium2 (Bass/Tile) DeformConv2d kernel.

kernel(x, offset, weight) -> np.ndarray [B, Cout, H, W] float32.
Data-parallel over batch: one SPMD Bass program per NeuronCore (8 cores).
Per core: bf16 x^T gather table in DRAM; DVE prep computes bilinear weights
(L128 layout) and pair-row gather indices (16-wrap layout, int16);
SWDGE dma_gather fetches 2-pixel channel rows; per-partition-scalar
multiplies + PE transpose-accumulate build val[c, j] in PSUM; per-tap
bf16 GEMM accumulates out[o, j] in PSUM over all 9 taps.
"""
import sys
import numpy as np

for _p in ("/opt/trn_rl_repo",):
    if _p not in sys.path:
        sys.path.insert(0, _p)

import concourse.bass as bass
import concourse.mybir as mybir
import concourse.tile as tile
from concourse import bacc
from concourse.masks import make_identity
from concourse.bass_utils import run_bass_kernel_spmd



f32 = mybir.dt.float32
bf16 = mybir.dt.bfloat16
i32 = mybir.dt.int32
i16 = mybir.dt.int16
Alu = mybir.AluOpType
P = 128


def build_dcn(C=256, Cout=256, H=64, W=64, KH=3, KW=3, PAD=1, CHUNK_JT=8,
              debug_prep=False, cast_round=True):
    HW = H * W
    S = HW // P
    SW = HW // 16
    NT = KH * KW
    CB = C // P
    MB = Cout // P
    assert S % CHUNK_JT == 0
    n_chunks = S // CHUNK_JT
    JC = CHUNK_JT * P
    NNB = (JC + 511) // 512
    FBIAS = 4.0 * max(H, W)
    # HW f32->int cast is round-nearest-even; CoreSim models truncation.
    FADD = FBIAS - (0.5 if cast_round else 0.0)

    nc = bacc.Bacc("TRN2", target_bir_lowering=False, debug=False)

    xt = nc.declare_dram_parameter("xt", [HW, C], f32, isOutput=False)
    offy = nc.declare_dram_parameter("offy", [P, NT, S], f32, isOutput=False)
    offx = nc.declare_dram_parameter("offx", [P, NT, S], f32, isOutput=False)
    byc = nc.declare_dram_parameter("byc", [P, NT, S], f32, isOutput=False)
    bxc = nc.declare_dram_parameter("bxc", [P, NT, S], f32, isOutput=False)
    offyW = nc.declare_dram_parameter("offyW", [P, NT, SW], f32, isOutput=False)
    offxW = nc.declare_dram_parameter("offxW", [P, NT, SW], f32, isOutput=False)
    bycW = nc.declare_dram_parameter("bycW", [P, NT, SW], f32, isOutput=False)
    bxcW = nc.declare_dram_parameter("bxcW", [P, NT, SW], f32, isOutput=False)
    wt = nc.declare_dram_parameter("wt", [P, NT, CB, Cout], f32, isOutput=False)
    out = nc.declare_dram_parameter("out", [Cout, HW], f32, isOutput=True)
    if debug_prep:
        dbg_w = nc.declare_dram_parameter("dbg_w", [4, P, NT, S], f32, isOutput=True)
        dbg_iA = nc.declare_dram_parameter("dbg_iA", [P, NT, SW], i32, isOutput=True)
        dbg_iB = nc.declare_dram_parameter("dbg_iB", [P, NT, SW], i32, isOutput=True)
        dbg_g = nc.declare_dram_parameter("dbg_g", [P, CHUNK_JT, 2 * C], f32, isOutput=True)
        dbg_v = nc.declare_dram_parameter("dbg_v", [P, CB, CHUNK_JT * P], f32, isOutput=True)

    xtb = nc.dram_tensor("xtb", [HW + 2, C], bf16)

    with tile.TileContext(nc) as tc:
        with tc.tile_pool(name="persist", bufs=1) as pe_pool:
            # persistent tiles
            wtb = pe_pool.tile([P, NT, CB, Cout], bf16, name="wtb")
            ident = pe_pool.tile([P, P], bf16, name="ident")
            w00 = pe_pool.tile([P, NT, S], f32, name="w00")
            w01 = pe_pool.tile([P, NT, S], f32, name="w01")
            w10 = pe_pool.tile([P, NT, S], f32, name="w10")
            w11 = pe_pool.tile([P, NT, S], f32, name="w11")
            idxA16 = pe_pool.tile([P, NT, SW], i16, name="idxA16")
            idxB16 = pe_pool.tile([P, NT, SW], i16, name="idxB16")

            make_identity(nc, ident[:])

            # ---- phase 0: xtb build + weight load (transient scratch)
            with tc.tile_pool(name="ph0", bufs=1) as s0:
                R = HW // P
                xt_sb = s0.tile([P, R * C], f32, name="xt_sb")
                nc.sync.dma_start(
                    out=xt_sb[:], in_=xt[:].rearrange("(p r) c -> p (r c)", p=P)
                )
                xt_bf = s0.tile([P, R * C], bf16, name="xt_bf")
                nc.vector.tensor_copy(out=xt_bf[:], in_=xt_sb[:])
                nc.sync.dma_start(
                    out=xtb[: HW].rearrange("(p r) c -> p (r c)", p=P), in_=xt_bf[:]
                )
                zpad = s0.tile([2, C], bf16, name="zpad")
                nc.vector.memset(zpad[:], 0.0)
                nc.sync.dma_start(out=xtb[HW : HW + 2], in_=zpad[:])
                wt_sb = s0.tile([P, NT * CB * Cout], f32, name="wt_sb")
                nc.sync.dma_start(
                    out=wt_sb[:], in_=wt[:].rearrange("p k b o -> p (k b o)")
                )
                nc.vector.tensor_copy(
                    out=wtb[:].rearrange("p k b o -> p (k b o)"), in_=wt_sb[:]
                )

            # ---- phase 1: L128 chain -> bilinear weights
            with tc.tile_pool(name="ph1", bufs=1) as sp:
                _ctr = [0]

                def newt(nm=None):
                    if nm is None:
                        _ctr[0] += 1
                        nm = f"pt{_ctr[0]}"
                    return sp.tile([P, NT, S], f32, name=nm)

                oy = newt("oy"); nc.sync.dma_start(out=oy[:], in_=offy[:])
                ox = newt("ox"); nc.sync.dma_start(out=ox[:], in_=offx[:])
                by = newt("by"); nc.sync.dma_start(out=by[:], in_=byc[:])
                bx = newt("bx"); nc.sync.dma_start(out=bx[:], in_=bxc[:])

                def tt(a, b, op):
                    o = newt()
                    nc.vector.tensor_tensor(out=o[:], in0=a[:], in1=b[:], op=op)
                    return o

                def ts(a, s1, op0, s2=None, op1=None):
                    o = newt()
                    if s2 is None:
                        nc.vector.tensor_scalar(
                            out=o[:], in0=a[:], scalar1=float(s1), scalar2=None,
                            op0=op0,
                        )
                    else:
                        nc.vector.tensor_scalar(
                            out=o[:], in0=a[:], scalar1=float(s1), scalar2=float(s2),
                            op0=op0, op1=op1,
                        )
                    return o

                py = tt(oy, by, Alu.add)
                px = tt(ox, bx, Alu.add)

                def floor_(v):
                    vb = ts(v, FADD, Alu.add)
                    vi = sp.tile([P, NT, S], i32, name=f"vi{_ctr[0]}")
                    nc.vector.tensor_copy(out=vi[:], in_=vb[:])
                    vf = newt()
                    nc.vector.tensor_copy(out=vf[:], in_=vi[:])
                    return ts(vf, FBIAS, Alu.subtract)

                y0 = floor_(py)
                x0 = floor_(px)
                ly = tt(py, y0, Alu.subtract)
                lx = tt(px, x0, Alu.subtract)

                def rng_mask(v, lo, hi):
                    a = ts(v, lo, Alu.is_ge)
                    b = ts(v, hi, Alu.is_le)
                    return tt(a, b, Alu.mult)

                my0 = rng_mask(y0, 0.0, H - 1)
                my1 = rng_mask(y0, -1.0, H - 2)
                mx0 = rng_mask(x0, 0.0, W - 1)
                mx1 = rng_mask(x0, -1.0, W - 2)

                one_m_ly = ts(ly, -1.0, Alu.mult, 1.0, Alu.add)
                one_m_lx = ts(lx, -1.0, Alu.mult, 1.0, Alu.add)
                vy0 = tt(one_m_ly, my0, Alu.mult)
                vy1 = tt(ly, my1, Alu.mult)
                ax0 = tt(one_m_lx, mx0, Alu.mult)
                ax1 = tt(lx, mx1, Alu.mult)

                sx = ts(x0, 0.0, Alu.max, W - 2, Alu.min)
                tsh = tt(x0, sx, Alu.subtract)
                e0 = ts(tsh, 0.0, Alu.is_equal)
                em1 = ts(tsh, -1.0, Alu.is_equal)
                e1 = ts(tsh, 1.0, Alu.is_equal)

                u0 = tt(tt(ax0, e0, Alu.mult), tt(ax1, em1, Alu.mult), Alu.add)
                u1 = tt(tt(ax0, e1, Alu.mult), tt(ax1, e0, Alu.mult), Alu.add)

                nc.vector.tensor_tensor(out=w00[:], in0=vy0[:], in1=u0[:], op=Alu.mult)
                nc.vector.tensor_tensor(out=w01[:], in0=vy0[:], in1=u1[:], op=Alu.mult)
                nc.vector.tensor_tensor(out=w10[:], in0=vy1[:], in1=u0[:], op=Alu.mult)
                nc.vector.tensor_tensor(out=w11[:], in0=vy1[:], in1=u1[:], op=Alu.mult)

            # ---- phase 2: W16 chain -> gather indices (per-tap to bound SBUF)
            with tc.tile_pool(name="ph2", bufs=1) as sq:
                _c2 = [0]

                def newq(nm=None):
                    if nm is None:
                        _c2[0] += 1
                        nm = f"qt{_c2[0]}"
                    return sq.tile([P, SW], f32, name=nm)

                def qtt(a, b, op, o=None):
                    o = o or newq()
                    nc.vector.tensor_tensor(out=o[:], in0=a[:], in1=b[:], op=op)
                    return o

                def qts(a, s1, op0, s2=None, op1=None, o=None):
                    o = o or newq()
                    if s2 is None:
                        nc.vector.tensor_scalar(
                            out=o[:], in0=a[:], scalar1=float(s1), scalar2=None,
                            op0=op0,
                        )
                    else:
                        nc.vector.tensor_scalar(
                            out=o[:], in0=a[:], scalar1=float(s1), scalar2=float(s2),
                            op0=op0, op1=op1,
                        )
                    return o

                oyq = newq("oyq"); oxq = newq("oxq")
                byq = newq("byq"); bxq = newq("bxq")
                pyq = newq("pyq"); pxq = newq("pxq")
                viq = sq.tile([P, SW], i32, name="viq")

                def qfloor(v, o=None):
                    vb = qts(v, FADD, Alu.add)
                    nc.vector.tensor_copy(out=viq[:], in_=vb[:])
                    nc.vector.tensor_copy(out=vb[:], in_=viq[:])
                    return qts(vb, FBIAS, Alu.subtract, o=o)

                y0q = newq("y0q"); x0q = newq("x0q")
                sxq = newq("sxq"); yc0q = newq("yc0q"); yc1q = newq("yc1q")
                iAf = newq("iAf"); iBf = newq("iBf")

                for k in range(NT):
                    nc.sync.dma_start(out=oyq[:], in_=offyW[:, k, :])
                    nc.sync.dma_start(out=oxq[:], in_=offxW[:, k, :])
                    nc.sync.dma_start(out=byq[:], in_=bycW[:, k, :])
                    nc.sync.dma_start(out=bxq[:], in_=bxcW[:, k, :])
                    qtt(oyq, byq, Alu.add, o=pyq)
                    qtt(oxq, bxq, Alu.add, o=pxq)
                    qfloor(pyq, o=y0q)
                    qfloor(pxq, o=x0q)
                    qts(x0q, 0.0, Alu.max, W - 2, Alu.min, o=sxq)
                    qts(y0q, 0.0, Alu.max, H - 1, Alu.min, o=yc0q)
                    t1 = qts(y0q, 1.0, Alu.add, 0.0, Alu.max)
                    qts(t1, H - 1, Alu.min, o=yc1q)
                    nc.vector.scalar_tensor_tensor(
                        out=iAf[:], in0=yc0q[:], scalar=float(W), in1=sxq[:],
                        op0=Alu.mult, op1=Alu.add,
                    )
                    nc.vector.scalar_tensor_tensor(
                        out=iBf[:], in0=yc1q[:], scalar=float(W), in1=sxq[:],
                        op0=Alu.mult, op1=Alu.add,
                    )
                    nc.vector.tensor_copy(out=idxA16[:, k, :], in_=iAf[:])
                    nc.vector.tensor_copy(out=idxB16[:, k, :], in_=iBf[:])

            if debug_prep:
                with tc.tile_pool(name="dbg", bufs=1) as dpool:
                    for wi, wv in enumerate((w00, w01, w10, w11)):
                        nc.sync.dma_start(out=dbg_w[wi], in_=wv[:])
                    dA = dpool.tile([P, NT, SW], i32, name="dA")
                    nc.vector.tensor_copy(out=dA[:], in_=idxA16[:])
                    nc.sync.dma_start(out=dbg_iA[:], in_=dA[:])
                    dB = dpool.tile([P, NT, SW], i32, name="dB")
                    nc.vector.tensor_copy(out=dB[:], in_=idxB16[:])
                    nc.sync.dma_start(out=dbg_iB[:], in_=dB[:])

            # overlapping-pair view of the bf16 table: row i covers elements
            # [C*i, C*i + 2C) — dma_gather elem_step=C, elem_size=2C.
            xtb_pairs = bass.AP(xtb[:].tensor, 0, [[C, HW], [1, 2 * C]])

            # ---- main loop
            with (
                tc.tile_pool(name="gather", bufs=2) as g_pool,
                tc.tile_pool(name="prod", bufs=2) as pr_pool,
                tc.tile_pool(name="vout", bufs=2) as v_pool,
                tc.tile_pool(name="obuf", bufs=2) as o_pool,
                tc.tile_pool(name="psum_out", bufs=1, space="PSUM") as pso_pool,
                tc.tile_pool(name="psum_val", bufs=1, space="PSUM") as psv_pool,
            ):
                SWC = JC // 16  # idx slots per chunk
                for ch in range(n_chunks):
                    out_ps = [
                        pso_pool.tile([P, JC], f32, space="PSUM", name=f"out_ps{_m}")
                        for _m in range(MB)
                    ]
                    for k in range(NT):
                        gA = g_pool.tile([P, CHUNK_JT, 2 * C], bf16, name="gA")
                        gB = g_pool.tile([P, CHUNK_JT, 2 * C], bf16, name="gB")
                        isl = slice(ch * SWC, (ch + 1) * SWC)
                        nc.gpsimd.dma_gather(
                            gA[:], xtb_pairs, idxA16[:, k, isl], JC, JC, 2 * C,
                            elem_step=C,
                        )
                        nc.gpsimd.dma_gather(
                            gB[:], xtb_pairs, idxB16[:, k, isl], JC, JC, 2 * C,
                            elem_step=C,
                        )
                        if debug_prep and ch == 0 and k == 0:
                            dg = pr_pool.tile([P, CHUNK_JT, 2 * C], f32, name="dg")
                            nc.vector.tensor_copy(out=dg[:], in_=gA[:])
                            nc.sync.dma_start(out=dbg_g[:], in_=dg[:])
                        val_ps = [
                            psv_pool.tile([P, JC], f32, space="PSUM",
                                          name=f"val_ps{_c}")
                            for _c in range(CB)
                        ]
                        for jt in range(CHUNK_JT):
                            s_idx = ch * CHUNK_JT + jt
                            pr = pr_pool.tile([P, 4, C], bf16, name="pr")
                            pieces = [
                                (gA, slice(0, C), w00),
                                (gA, slice(C, 2 * C), w01),
                                (gB, slice(0, C), w10),
                                (gB, slice(C, 2 * C), w11),
                            ]
                            for n, (g, csl, wv) in enumerate(pieces):
                                nc.vector.tensor_scalar(
                                    out=pr[:, n, :], in0=g[:, jt, csl],
                                    scalar1=wv[:, k, s_idx : s_idx + 1],
                                    scalar2=None, op0=Alu.mult,
                                )
                            for n in range(4):
                                for cb in range(CB):
                                    nc.tensor.matmul(
                                        out=val_ps[cb][:, jt * P : (jt + 1) * P],
                                        lhsT=pr[:, n, cb * P : (cb + 1) * P],
                                        rhs=ident[:],
                                        start=(n == 0),
                                        stop=(n == 3),
                                    )
                        vsb = v_pool.tile([P, CB, JC], bf16, name="vsb")
                        for cb in range(CB):
                            nc.vector.tensor_copy(out=vsb[:, cb, :], in_=val_ps[cb][:])
                        if debug_prep and ch == 0 and k == 0:
                            dv = v_pool.tile([P, CB, JC], f32, name="dv")
                            nc.vector.tensor_copy(out=dv[:], in_=vsb[:])
                            nc.sync.dma_start(out=dbg_v[:], in_=dv[:])
                        for mb in range(MB):
                            for cb in range(CB):
                                for nb in range(NNB):
                                    nsl = slice(nb * 512, min((nb + 1) * 512, JC))
                                    nc.tensor.matmul(
                                        out=out_ps[mb][:, nsl],
                                        lhsT=wtb[:, k, cb, mb * P : (mb + 1) * P],
                                        rhs=vsb[:, cb, nsl],
                                        start=(k == 0 and cb == 0),
                                        stop=(k == NT - 1 and cb == CB - 1),
                                    )
                    for mb in range(MB):
                        ob = o_pool.tile([P, JC], f32, name="ob")
                        nc.vector.tensor_copy(out=ob[:], in_=out_ps[mb][:])
                        nc.sync.dma_start(
                            out=out[mb * P : (mb + 1) * P, ch * JC : (ch + 1) * JC],
                            in_=ob[:],
                        )

    nc.compile()
    return nc


def host_prep(x_b, offset_b, weight, H, W, KH, KW, PAD):
    """Per-core input map from one batch slice (numpy, f32)."""
    C = x_b.shape[0]
    Cout = weight.shape[0]
    HW = H * W
    S = HW // P
    SW = HW // 16
    NT = KH * KW
    CB = C // P
    xt = np.ascontiguousarray(x_b.reshape(C, HW).T).astype(np.float32)
    off = offset_b.reshape(NT, 2, HW)
    j = np.arange(HW)
    ks = np.arange(NT)
    byv = (j[None, :] // W - PAD + (ks // KW)[:, None]).astype(np.float32)  # [k, j]
    bxv = (j[None, :] % W - PAD + (ks % KW)[:, None]).astype(np.float32)

    def l128(a):  # [k, j] -> [p, k, s], j = 128*s + p
        return np.ascontiguousarray(a.reshape(NT, S, P).transpose(2, 0, 1)).astype(np.float32)

    def w16(a):  # [k, j] -> [q + 16g, k, s], j = 16*s + q, replicated over g
        b = a.reshape(NT, SW, 16).transpose(2, 0, 1)  # [q, k, s]
        return np.ascontiguousarray(np.tile(b, (8, 1, 1))).astype(np.float32)

    wr = weight.reshape(Cout, C, NT)
    wtv = wr.reshape(Cout, CB, P, NT).transpose(2, 3, 1, 0)
    return {
        "xt": xt,
        "offy": l128(off[:, 0]), "offx": l128(off[:, 1]),
        "byc": l128(byv), "bxc": l128(bxv),
        "offyW": w16(off[:, 0]), "offxW": w16(off[:, 1]),
        "bycW": w16(byv), "bxcW": w16(bxv),
        "wt": np.ascontiguousarray(wtv).astype(np.float32),
    }


_NC_CACHE = {}


def _get_nc(key, **kw):
    if key not in _NC_CACHE:
        _NC_CACHE[key] = build_dcn(**kw)
    return _NC_CACHE[key]


def kernel(x, offset, weight):
    x = np.asarray(x, dtype=np.float32)
    offset = np.asarray(offset, dtype=np.float32)
    weight = np.asarray(weight, dtype=np.float32)
    B, C, H, W = x.shape
    Cout = weight.shape[0]
    KH, KW = weight.shape[2], weight.shape[3]
    PAD = 1
    assert B == 8 and C % 128 == 0 and Cout % 128 == 0
    nc = _get_nc((C, Cout, H, W, KH, KW), C=C, Cout=Cout, H=H, W=W,
                 KH=KH, KW=KW, PAD=PAD, CHUNK_JT=8)
    in_maps = [host_prep(x[b], offset[b], weight, H, W, KH, KW, PAD)
               for b in range(B)]
    res = run_bass_kernel_spmd(nc, in_maps, list(range(B)))
    out = np.stack([res.results[b]["out"].reshape(Cout, H, W) for b in range(B)])
    return out.astype(np.float32)



# revision 3
# speedup vs baseline: 1.2388x; 1.2388x over previous
"""Self-contained Trainium2 (Bass/Tile) DeformConv2d kernel.

kernel(x, offset, weight) -> np.ndarray [B, Cout, H, W] float32.
Data-parallel over batch: one SPMD Bass program per NeuronCore (8 cores).

Per core (one image): the bf16 x^T table lives in DRAM and is gathered with
4 int16 indices per (tap, pixel) — one per bilinear corner — laid out so the
4 corners of 32 pixels fill the 128 gather partitions (partition = 32*n + q).
DVE computes the 4 bilinear corner weights on-chip (L128 layout), 16 small
SBUF->SBUF DMAs shuffle them into the corner-stacked layout, and one narrow
[128,32] TensorScalarPtr per (tap, 32-px group) builds M = diag(w) @ K.
A single PE matmul per (tap, group, c-half) then performs
scale+combine+transpose+sum in one shot: valT[c,j] = sum_p g[p,c] * M[p,j].
The per-tap GEMM accumulates out[o, px] over taps in PSUM.
"""
import sys
import numpy as np

for _p in ("/opt/trn_rl_repo",):
    if _p not in sys.path:
        sys.path.insert(0, _p)

import concourse.bass as bass
import concourse.mybir as mybir
import concourse.tile as tile
from concourse import bacc
from concourse.bass_utils import run_bass_kernel_spmd

try:
    from ml_dtypes import bfloat16 as np_bf16
except ImportError:  # jax ships ml_dtypes; fall back via jax.numpy
    import jax.numpy as _jnp
    np_bf16 = _jnp.bfloat16

f32 = mybir.dt.float32
bf16 = mybir.dt.bfloat16
i32 = mybir.dt.int32
i16 = mybir.dt.int16
Alu = mybir.AluOpType
P = 128


def build_dcn(C=256, Cout=256, H=64, W=64, NT=9, NSTRIPE=4):
    HW = H * W
    S = HW // P              # 32 (L128 free index)
    CB = C // P              # 2
    MB = Cout // P           # 2
    SPX = HW // NSTRIPE      # 1024 pixels per stripe
    NSL = SPX // P           # 8 s_locals per stripe
    NSUB = NSL // 4          # 2 PSUM substripes (512 px each)
    NIDX = 4 * SPX           # 4096 gather slots per (tap, stripe)
    SW = NIDX // 16          # 256 idx free slots (16-wrap)
    FBIAS = 4.0 * max(H, W)
    FADD = FBIAS - 0.5       # HW f32->int cast is round-nearest-even

    nc = bacc.Bacc("TRN2", target_bir_lowering=False, debug=False)

    xtab = nc.declare_dram_parameter("xtab", [HW, C], bf16, isOutput=False)
    idxp = nc.declare_dram_parameter("idxp", [P, NT, NSTRIPE, SW], i16, isOutput=False)
    offy = nc.declare_dram_parameter("offy", [P, NT, S], f32, isOutput=False)
    offx = nc.declare_dram_parameter("offx", [P, NT, S], f32, isOutput=False)
    byc = nc.declare_dram_parameter("byc", [P, NT, S], f32, isOutput=False)
    bxc = nc.declare_dram_parameter("bxc", [P, NT, S], f32, isOutput=False)
    wt = nc.declare_dram_parameter("wt", [P, NT, CB, Cout], bf16, isOutput=False)
    kmat = nc.declare_dram_parameter("kmat", [P, 32], bf16, isOutput=False)
    out = nc.declare_dram_parameter("out", [Cout, HW], bf16, isOutput=True)

    xtab_ap = bass.AP(xtab[:].tensor, 0, [[C, HW], [1, C]])

    with tile.TileContext(nc) as tc:
        with tc.tile_pool(name="persist", bufs=1) as pp:
            idx_sb = pp.tile([P, NT, NSTRIPE, SW], i16, name="idx_sb")
            nc.sync.dma_start(out=idx_sb[:], in_=idxp[:])
            wtb = pp.tile([P, NT, CB, Cout], bf16, name="wtb")
            nc.sync.dma_start(out=wtb[:], in_=wt[:])
            ksb = pp.tile([P, 32], bf16, name="ksb")
            nc.sync.dma_start(out=ksb[:], in_=kmat[:])
            # corner weights in corner-stacked layout: [p=32n+q, d, k, s]
            wcol = pp.tile([P, 4, NT, S], f32, name="wcol")

            # ---- phase 1: bilinear corner weights (L128 layout: px = 128s+p)
            with tc.tile_pool(name="ph1", bufs=1) as sp:
                _ctr = [0]

                def newt(nm=None):
                    if nm is None:
                        _ctr[0] += 1
                        nm = f"pt{_ctr[0]}"
                    return sp.tile([P, NT, S], f32, name=nm)

                oy = newt("oy"); nc.sync.dma_start(out=oy[:], in_=offy[:])
                ox = newt("ox"); nc.sync.dma_start(out=ox[:], in_=offx[:])
                by = newt("by"); nc.sync.dma_start(out=by[:], in_=byc[:])
                bx = newt("bx"); nc.sync.dma_start(out=bx[:], in_=bxc[:])

                def tt(a, b, op, o=None):
                    o = o or newt()
                    nc.vector.tensor_tensor(out=o[:], in0=a[:], in1=b[:], op=op)
                    return o

                def ts(a, s1, op0, s2=None, op1=None):
                    o = newt()
                    if s2 is None:
                        nc.vector.tensor_scalar(
                            out=o[:], in0=a[:], scalar1=float(s1), scalar2=None,
                            op0=op0,
                        )
                    else:
                        nc.vector.tensor_scalar(
                            out=o[:], in0=a[:], scalar1=float(s1), scalar2=float(s2),
                            op0=op0, op1=op1,
                        )
                    return o

                py = tt(oy, by, Alu.add)
                px = tt(ox, bx, Alu.add)

                def floor_(v):
                    vb = ts(v, FADD, Alu.add)
                    vi = sp.tile([P, NT, S], i32, name=f"vi{_ctr[0]}")
                    nc.vector.tensor_copy(out=vi[:], in_=vb[:])
                    vf = newt()
                    nc.vector.tensor_copy(out=vf[:], in_=vi[:])
                    return ts(vf, FBIAS, Alu.subtract)

                y0 = floor_(py)
                x0 = floor_(px)
                ly = tt(py, y0, Alu.subtract)
                lx = tt(px, x0, Alu.subtract)

                def rng_mask(v, lo, hi):
                    a = ts(v, lo, Alu.is_ge)
                    b = ts(v, hi, Alu.is_le)
                    return tt(a, b, Alu.mult)

                my0 = rng_mask(y0, 0.0, H - 1)
                my1 = rng_mask(y0, -1.0, H - 2)
                mx0 = rng_mask(x0, 0.0, W - 1)
                mx1 = rng_mask(x0, -1.0, W - 2)

                one_m_ly = ts(ly, -1.0, Alu.mult, 1.0, Alu.add)
                one_m_lx = ts(lx, -1.0, Alu.mult, 1.0, Alu.add)
                vy0 = tt(one_m_ly, my0, Alu.mult)
                vy1 = tt(ly, my1, Alu.mult)
                ux0 = tt(one_m_lx, mx0, Alu.mult)
                ux1 = tt(lx, mx1, Alu.mult)

                w4 = [tt(vy0, ux0, Alu.mult), tt(vy0, ux1, Alu.mult),
                      tt(vy1, ux0, Alu.mult), tt(vy1, ux1, Alu.mult)]

                # shuffle into corner-stacked layout via 16 tiny SBUF->SBUF
                # DMAs: wcol[32n+q, d, k, s] = w_n[32d+q, k, s]
                for n in range(4):
                    for d in range(4):
                        nc.sync.dma_start(
                            out=wcol[32 * n:32 * (n + 1), d],
                            in_=w4[n][32 * d:32 * (d + 1)],
                        )

            # ---- main loop
            with (
                tc.tile_pool(name="gather", bufs=2) as g_pool,
                tc.tile_pool(name="mtiles", bufs=2) as m_pool,
                tc.tile_pool(name="vout", bufs=2) as v_pool,
                tc.tile_pool(name="obuf", bufs=2) as o_pool,
                tc.tile_pool(name="psum_out", bufs=1, space="PSUM") as pso_pool,
                tc.tile_pool(name="psum_val", bufs=2, space="PSUM") as psv_pool,
            ):
                for st in range(NSTRIPE):
                    out_ps = [
                        pso_pool.tile([P, SPX], f32, space="PSUM", name=f"ops{m}")
                        for m in range(MB)
                    ]
                    for k in range(NT):
                        gbuf = g_pool.tile([P, NSL * 4, C], bf16, name="gbuf")
                        nc.gpsimd.dma_gather(
                            gbuf[:], xtab_ap, idx_sb[:, k, st, :], NIDX, NIDX, C,
                        )
                        mall = m_pool.tile([P, NSL * 4, 32], bf16, name="mall")
                        for sl in range(NSL):
                            for d in range(4):
                                g = 4 * sl + d
                                nc.vector.tensor_scalar(
                                    out=mall[:, g, :], in0=ksb[:],
                                    scalar1=wcol[:, d, k, 8 * st + sl:8 * st + sl + 1],
                                    scalar2=None, op0=Alu.mult,
                                )
                        for sub in range(NSUB):
                            val_ps = [
                                psv_pool.tile([P, P * 4], f32, space="PSUM",
                                              name=f"vps{cb}")
                                for cb in range(CB)
                            ]
                            for slq in range(4):
                                sl = sub * 4 + slq
                                for d in range(4):
                                    g = 4 * sl + d
                                    col = slq * P + 32 * d
                                    for cb in range(CB):
                                        nc.tensor.matmul(
                                            out=val_ps[cb][:, col:col + 32],
                                            lhsT=gbuf[:, g, cb * P:(cb + 1) * P],
                                            rhs=mall[:, g, :],
                                            start=True, stop=True,
                                        )
                            vsb = v_pool.tile([P, CB, P * 4], bf16, name="vsb")
                            nc.vector.tensor_copy(out=vsb[:, 0], in_=val_ps[0][:])
                            nc.scalar.copy(out=vsb[:, 1], in_=val_ps[1][:])
                            for mb in range(MB):
                                for cb in range(CB):
                                    nc.tensor.matmul(
                                        out=out_ps[mb][:, sub * 512:(sub + 1) * 512],
                                        lhsT=wtb[:, k, cb, mb * P:(mb + 1) * P],
                                        rhs=vsb[:, cb],
                                        start=(k == 0 and cb == 0),
                                        stop=(k == NT - 1 and cb == CB - 1),
                                    )
                    for mb in range(MB):
                        ob = o_pool.tile([P, SPX], bf16, name="ob")
                        nc.scalar.copy(out=ob[:], in_=out_ps[mb][:])
                        nc.sync.dma_start(
                            out=out[mb * P:(mb + 1) * P, st * SPX:(st + 1) * SPX],
                            in_=ob[:],
                        )

    nc.compile()
    return nc


def host_prep(x_b, offset_b, weight, H, W, KH, KW, PAD):
    """Per-core input map from one batch slice (numpy, f32)."""
    C = x_b.shape[0]
    Cout = weight.shape[0]
    HW = H * W
    S = HW // P
    NT = KH * KW
    CB = C // P
    NSTRIPE = 4
    SPX = HW // NSTRIPE
    NIDX = 4 * SPX
    SW = NIDX // 16

    xt = np.ascontiguousarray(x_b.reshape(C, HW).T).astype(np_bf16)
    off = offset_b.reshape(NT, 2, HW)
    j = np.arange(HW)
    ks = np.arange(NT)
    byv = (j[None, :] // W - PAD + (ks // KW)[:, None]).astype(np.float32)  # [k, j]
    bxv = (j[None, :] % W - PAD + (ks % KW)[:, None]).astype(np.float32)

    def l128(a):  # [k, j] -> [p, k, s], j = 128*s + p
        return np.ascontiguousarray(a.reshape(NT, S, P).transpose(2, 0, 1)).astype(np.float32)

    # per-corner clamped gather rows, mirroring the device f32 floor trick
    py = off[:, 0].astype(np.float32) + byv
    px = off[:, 1].astype(np.float32) + bxv
    FADD = np.float32(4.0 * max(H, W) - 0.5)
    y0 = np.rint(py + FADD).astype(np.int64) - int(4.0 * max(H, W))
    x0 = np.rint(px + FADD).astype(np.int64) - int(4.0 * max(H, W))
    rq = np.zeros((4, NT, HW), np.int64)
    for n in range(4):
        yn = np.clip(y0 + (n >> 1), 0, H - 1)
        xn = np.clip(x0 + (n & 1), 0, W - 1)
        rq[n] = yn * W + xn

    # slot order per (k, stripe): i = 128*(4*sl+d) + 32*n + q,
    # pixel = stripe*SPX + 128*sl + 32*d + q
    sl_i = np.arange(NIDX) // 512          # within-stripe s_local
    d_i = (np.arange(NIDX) // 128) % 4
    n_i = (np.arange(NIDX) % 128) // 32
    q_i = np.arange(NIDX) % 32
    pxl = 128 * sl_i + 32 * d_i + q_i      # [NIDX]
    idx = np.zeros((P, NT, NSTRIPE, SW), np.int16)
    for st in range(NSTRIPE):
        rows = rq[n_i, :, st * SPX + pxl].astype(np.int16)  # [NIDX, NT]
        wrap = rows.reshape(SW, 16, NT).transpose(1, 0, 2)  # [16, SW, NT]
        idx[:, :, st, :] = np.tile(
            wrap.transpose(0, 2, 1), (8, 1, 1))            # [128, NT, SW]

    wr = weight.reshape(Cout, C, NT)
    wtv = wr.reshape(Cout, CB, P, NT).transpose(2, 3, 1, 0)  # [p, k, cb, o]
    kmat = (np.arange(P)[:, None] % 32 == np.arange(32)[None, :])

    return {
        "xtab": xt,
        "idxp": idx,
        "offy": l128(off[:, 0]), "offx": l128(off[:, 1]),
        "byc": l128(byv), "bxc": l128(bxv),
        "wt": np.ascontiguousarray(wtv).astype(np_bf16),
        "kmat": kmat.astype(np_bf16),
    }


_NC_CACHE = {}


def _get_nc(key, **kw):
    if key not in _NC_CACHE:
        _NC_CACHE[key] = build_dcn(**kw)
    return _NC_CACHE[key]


def kernel(x, offset, weight):
    x = np.asarray(x, dtype=np.float32)
    offset = np.asarray(offset, dtype=np.float32)
    weight = np.asarray(weight, dtype=np.float32)
    B, C, H, W = x.shape
    Cout = weight.shape[0]
    KH, KW = weight.shape[2], weight.shape[3]
    PAD = 1
    assert B == 8 and C % 128 == 0 and Cout % 128 == 0
    nc = _get_nc((C, Cout, H, W, KH, KW), C=C, Cout=Cout, H=H, W=W,
                 NT=KH * KW)
    in_maps = [host_prep(x[b], offset[b], weight, H, W, KH, KW, PAD)
               for b in range(B)]
    res = run_bass_kernel_spmd(nc, in_maps, list(range(B)))
    out = np.stack([res.results[b]["out"].astype(np.float32).reshape(Cout, H, W)
                    for b in range(B)])
    return out


# revision 4
# speedup vs baseline: 1.4920x; 1.2044x over previous
"""Self-contained Trainium2 (Bass/Tile) DeformConv2d kernel.

kernel(x, offset, weight) -> np.ndarray [B, Cout, H, W] float32.
Data-parallel over batch: one SPMD Bass program per NeuronCore (8 cores).

Per core (one image): the bf16 x^T table lives in DRAM and is gathered with
4 int16 indices per (tap, pixel) — one per bilinear corner — laid out so the
4 corners of 32 pixels fill the 128 gather partitions (partition = 32*n + q).
DVE computes the 4 bilinear corner weights on-chip (L128 layout), 16 small
SBUF->SBUF DMAs shuffle them into the corner-stacked layout, and one narrow
[128,32] TensorScalarPtr per (tap, 32-px group) builds M = diag(w) @ K.
A single PE matmul per (tap, group, c-half) then performs
scale+combine+transpose+sum in one shot: valT[c,j] = sum_p g[p,c] * M[p,j].
The per-tap GEMM accumulates out[o, px] over taps in PSUM.
"""
import sys
import numpy as np

for _p in ("/opt/trn_rl_repo",):
    if _p not in sys.path:
        sys.path.insert(0, _p)

import concourse.bass as bass
import concourse.mybir as mybir
import concourse.tile as tile
from concourse import bacc
from concourse.bass_utils import run_bass_kernel_spmd

try:
    from ml_dtypes import bfloat16 as np_bf16
except ImportError:  # jax ships ml_dtypes; fall back via jax.numpy
    import jax.numpy as _jnp
    np_bf16 = _jnp.bfloat16

f32 = mybir.dt.float32
bf16 = mybir.dt.bfloat16
i32 = mybir.dt.int32
i16 = mybir.dt.int16
Alu = mybir.AluOpType
P = 128


def build_dcn(C=256, Cout=256, H=64, W=64, NT=9, NSTRIPE=8):
    HW = H * W
    S = HW // P              # 32 (L128 free index; s = pixel // 128)
    CB = C // P              # 2
    MB = Cout // P           # 2
    SPX = HW // NSTRIPE      # 512 pixels per stripe
    NSL = SPX // P           # 4 s_locals per stripe
    NG = SPX // 32           # 16 groups of 32 px per stripe
    NIDX = 4 * SPX           # 2048 gather slots per (tap, stripe)
    SW = NIDX // 16          # 128 idx free slots (16-wrap)
    FBIAS = 4.0 * max(H, W)
    FADD = FBIAS - 0.5       # HW f32->int cast is round-nearest-even

    nc = bacc.Bacc("TRN2", target_bir_lowering=False, debug=False)

    xtab = nc.declare_dram_parameter("xtab", [HW, C], bf16, isOutput=False)
    idxp = nc.declare_dram_parameter("idxp", [NSTRIPE, P, NT, SW], i16,
                                     isOutput=False)
    offy = nc.declare_dram_parameter("offy", [P, NT, S], f32, isOutput=False)
    offx = nc.declare_dram_parameter("offx", [P, NT, S], f32, isOutput=False)
    byc = nc.declare_dram_parameter("byc", [P, NT, S], f32, isOutput=False)
    bxc = nc.declare_dram_parameter("bxc", [P, NT, S], f32, isOutput=False)
    wt = nc.declare_dram_parameter("wt", [P, NT, CB, Cout], bf16, isOutput=False)
    kmat = nc.declare_dram_parameter("kmat", [P, 32], bf16, isOutput=False)
    out = nc.declare_dram_parameter("out", [Cout, HW], bf16, isOutput=True)

    xtab_ap = bass.AP(xtab[:].tensor, 0, [[C, HW], [1, C]])

    with tile.TileContext(nc) as tc:
        with tc.tile_pool(name="persist", bufs=1) as pp:
            idx_sb = pp.tile([P, NSTRIPE, NT, SW], i16, name="idx_sb")
            # stripe-0 indices first so gathers start immediately
            nc.sync.dma_start(out=idx_sb[:, 0], in_=idxp[0])
            oy = pp.tile([P, NT, S], f32, name="oy")
            ox = pp.tile([P, NT, S], f32, name="ox")
            by = pp.tile([P, NT, S], f32, name="by")
            bx = pp.tile([P, NT, S], f32, name="bx")
            nc.sync.dma_start(out=oy[:], in_=offy[:])
            nc.sync.dma_start(out=ox[:], in_=offx[:])
            nc.sync.dma_start(out=by[:], in_=byc[:])
            nc.sync.dma_start(out=bx[:], in_=bxc[:])
            for st in range(1, NSTRIPE):
                nc.sync.dma_start(out=idx_sb[:, st], in_=idxp[st])
            wtb = pp.tile([P, NT, CB, Cout], bf16, name="wtb")
            nc.sync.dma_start(out=wtb[:], in_=wt[:])
            ksb = pp.tile([P, 32], bf16, name="ksb")
            nc.sync.dma_start(out=ksb[:], in_=kmat[:])
            # corner weights in corner-stacked layout: [p=32n+q, d, k, s]
            wcol = pp.tile([P, 4, NT, S], f32, name="wcol")

            with (
                tc.tile_pool(name="gather", bufs=6) as g_pool,
                tc.tile_pool(name="mtiles", bufs=4) as m_pool,
                tc.tile_pool(name="vout", bufs=3) as v_pool,
                tc.tile_pool(name="obuf", bufs=2) as o_pool,
                tc.tile_pool(name="psum_out", bufs=1, space="PSUM") as pso_pool,
                tc.tile_pool(name="psum_val", bufs=3, space="PSUM") as psv_pool,
            ):
                # ---- phase 1: bilinear corner weights (L128: px = 128s+p)
                with tc.tile_pool(name="ph1", bufs=1) as sp:
                    _ctr = [0]

                    def newt(nm=None):
                        if nm is None:
                            _ctr[0] += 1
                            nm = f"pt{_ctr[0]}"
                        return sp.tile([P, NT, S], f32, name=nm)

                    def tt(a, b, op):
                        o = newt()
                        nc.vector.tensor_tensor(out=o[:], in0=a[:], in1=b[:], op=op)
                        return o

                    def ts(a, s1, op0, s2=None, op1=None):
                        o = newt()
                        if s2 is None:
                            nc.vector.tensor_scalar(
                                out=o[:], in0=a[:], scalar1=float(s1),
                                scalar2=None, op0=op0,
                            )
                        else:
                            nc.vector.tensor_scalar(
                                out=o[:], in0=a[:], scalar1=float(s1),
                                scalar2=float(s2), op0=op0, op1=op1,
                            )
                        return o

                    py = tt(oy, by, Alu.add)
                    px = tt(ox, bx, Alu.add)

                    def floor_(v):
                        vb = ts(v, FADD, Alu.add)
                        vi = sp.tile([P, NT, S], i32, name=f"vi{_ctr[0]}")
                        nc.vector.tensor_copy(out=vi[:], in_=vb[:])
                        vf = newt()
                        nc.vector.tensor_copy(out=vf[:], in_=vi[:])
                        return ts(vf, FBIAS, Alu.subtract)

                    y0 = floor_(py)
                    x0 = floor_(px)
                    ly = tt(py, y0, Alu.subtract)
                    lx = tt(px, x0, Alu.subtract)

                    def rng_mask(v, lo, hi):
                        a = ts(v, lo, Alu.is_ge)
                        b = ts(v, hi, Alu.is_le)
                        return tt(a, b, Alu.mult)

                    my0 = rng_mask(y0, 0.0, H - 1)
                    my1 = rng_mask(y0, -1.0, H - 2)
                    mx0 = rng_mask(x0, 0.0, W - 1)
                    mx1 = rng_mask(x0, -1.0, W - 2)

                    one_m_ly = ts(ly, -1.0, Alu.mult, 1.0, Alu.add)
                    one_m_lx = ts(lx, -1.0, Alu.mult, 1.0, Alu.add)
                    vy0 = tt(one_m_ly, my0, Alu.mult)
                    vy1 = tt(ly, my1, Alu.mult)
                    ux0 = tt(one_m_lx, mx0, Alu.mult)
                    ux1 = tt(lx, mx1, Alu.mult)

                    w4 = [tt(vy0, ux0, Alu.mult), tt(vy0, ux1, Alu.mult),
                          tt(vy1, ux0, Alu.mult), tt(vy1, ux1, Alu.mult)]

                    # shuffle into corner-stacked layout via 16 tiny
                    # SBUF->SBUF DMAs: wcol[32n+q, d, k, s] = w_n[32d+q, k, s]
                    # (split across SP and Act queues to halve config time)
                    for n in range(4):
                        for d in range(4):
                            eng = nc.sync if (n % 2 == 0) else nc.scalar
                            eng.dma_start(
                                out=wcol[32 * n:32 * (n + 1), d],
                                in_=w4[n][32 * d:32 * (d + 1)],
                            )

                # ---- main loop
                for st in range(NSTRIPE):
                    out_ps = [
                        pso_pool.tile([P, SPX], f32, space="PSUM", name=f"ops{m}")
                        for m in range(MB)
                    ]
                    for k in range(NT):
                        gbuf = g_pool.tile([P, NG, C], bf16, name="gbuf")
                        nc.gpsimd.dma_gather(
                            gbuf[:], xtab_ap, idx_sb[:, st, k, :], NIDX, NIDX, C,
                        )
                        mall = m_pool.tile([P, NG, 32], bf16, name="mall")
                        for sl in range(NSL):
                            for d in range(4):
                                g = 4 * sl + d
                                s_g = NSL * st + sl
                                nc.vector.tensor_scalar(
                                    out=mall[:, g, :], in0=ksb[:],
                                    scalar1=wcol[:, d, k, s_g:s_g + 1],
                                    scalar2=None, op0=Alu.mult,
                                )
                        val_ps = [
                            psv_pool.tile([P, SPX], f32, space="PSUM",
                                          name=f"vps{cb}")
                            for cb in range(CB)
                        ]
                        for sl in range(NSL):
                            for d in range(4):
                                g = 4 * sl + d
                                col = sl * P + 32 * d
                                for cb in range(CB):
                                    nc.tensor.matmul(
                                        out=val_ps[cb][:, col:col + 32],
                                        lhsT=gbuf[:, g, cb * P:(cb + 1) * P],
                                        rhs=mall[:, g, :],
                                        start=True, stop=True,
                                    )
                        vsb = v_pool.tile([P, CB, SPX], bf16, name="vsb")
                        nc.vector.tensor_copy(out=vsb[:, 0], in_=val_ps[0][:])
                        nc.scalar.copy(out=vsb[:, 1], in_=val_ps[1][:])
                        for mb in range(MB):
                            for cb in range(CB):
                                nc.tensor.matmul(
                                    out=out_ps[mb][:],
                                    lhsT=wtb[:, k, cb, mb * P:(mb + 1) * P],
                                    rhs=vsb[:, cb],
                                    start=(k == 0 and cb == 0),
                                    stop=(k == NT - 1 and cb == CB - 1),
                                )
                    for mb in range(MB):
                        ob = o_pool.tile([P, SPX], bf16, name="ob")
                        nc.scalar.copy(out=ob[:], in_=out_ps[mb][:])
                        nc.sync.dma_start(
                            out=out[mb * P:(mb + 1) * P, st * SPX:(st + 1) * SPX],
                            in_=ob[:],
                        )

    nc.compile()
    return nc


def host_prep(x_b, offset_b, weight, H, W, KH, KW, PAD):
    """Per-core input map from one batch slice (numpy, f32)."""
    C = x_b.shape[0]
    Cout = weight.shape[0]
    HW = H * W
    S = HW // P
    NT = KH * KW
    CB = C // P
    NSTRIPE = 8
    SPX = HW // NSTRIPE
    NIDX = 4 * SPX
    SW = NIDX // 16

    xt = np.ascontiguousarray(x_b.reshape(C, HW).T).astype(np_bf16)
    off = offset_b.reshape(NT, 2, HW)
    j = np.arange(HW)
    ks = np.arange(NT)
    byv = (j[None, :] // W - PAD + (ks // KW)[:, None]).astype(np.float32)  # [k, j]
    bxv = (j[None, :] % W - PAD + (ks % KW)[:, None]).astype(np.float32)

    def l128(a):  # [k, j] -> [p, k, s], j = 128*s + p
        return np.ascontiguousarray(a.reshape(NT, S, P).transpose(2, 0, 1)).astype(np.float32)

    # per-corner clamped gather rows, mirroring the device f32 floor trick
    py = off[:, 0].astype(np.float32) + byv
    px = off[:, 1].astype(np.float32) + bxv
    FADD = np.float32(4.0 * max(H, W) - 0.5)
    y0 = np.rint(py + FADD).astype(np.int64) - int(4.0 * max(H, W))
    x0 = np.rint(px + FADD).astype(np.int64) - int(4.0 * max(H, W))
    rq = np.zeros((4, NT, HW), np.int64)
    for n in range(4):
        yn = np.clip(y0 + (n >> 1), 0, H - 1)
        xn = np.clip(x0 + (n & 1), 0, W - 1)
        rq[n] = yn * W + xn

    # slot order per (stripe, k): i = 128*(4*sl+d) + 32*n + q,
    # pixel = stripe*SPX + 128*sl + 32*d + q
    i_arr = np.arange(NIDX)
    sl_i = i_arr // 512
    d_i = (i_arr // 128) % 4
    n_i = (i_arr % 128) // 32
    q_i = i_arr % 32
    pxl = 128 * sl_i + 32 * d_i + q_i      # [NIDX]
    idx = np.zeros((NSTRIPE, P, NT, SW), np.int16)
    for st in range(NSTRIPE):
        rows = rq[n_i, :, st * SPX + pxl].astype(np.int16)  # [NIDX, NT]
        wrap = rows.reshape(SW, 16, NT).transpose(1, 2, 0)  # [16, NT, SW]
        idx[st] = np.tile(wrap, (8, 1, 1))                  # [128, NT, SW]

    wr = weight.reshape(Cout, C, NT)
    wtv = wr.reshape(Cout, CB, P, NT).transpose(2, 3, 1, 0)  # [p, k, cb, o]
    kmat = (np.arange(P)[:, None] % 32 == np.arange(32)[None, :])

    return {
        "xtab": xt,
        "idxp": idx,
        "offy": l128(off[:, 0]), "offx": l128(off[:, 1]),
        "byc": l128(byv), "bxc": l128(bxv),
        "wt": np.ascontiguousarray(wtv).astype(np_bf16),
        "kmat": kmat.astype(np_bf16),
    }


_NC_CACHE = {}


def _get_nc(key, **kw):
    if key not in _NC_CACHE:
        _NC_CACHE[key] = build_dcn(**kw)
    return _NC_CACHE[key]


def kernel(x, offset, weight):
    x = np.asarray(x, dtype=np.float32)
    offset = np.asarray(offset, dtype=np.float32)
    weight = np.asarray(weight, dtype=np.float32)
    B, C, H, W = x.shape
    Cout = weight.shape[0]
    KH, KW = weight.shape[2], weight.shape[3]
    PAD = 1
    assert B == 8 and C % 128 == 0 and Cout % 128 == 0
    nc = _get_nc((C, Cout, H, W, KH, KW), C=C, Cout=Cout, H=H, W=W,
                 NT=KH * KW)
    in_maps = [host_prep(x[b], offset[b], weight, H, W, KH, KW, PAD)
               for b in range(B)]
    res = run_bass_kernel_spmd(nc, in_maps, list(range(B)))
    out = np.stack([res.results[b]["out"].astype(np.float32).reshape(Cout, H, W)
                    for b in range(B)])
    return out


# revision 6
# speedup vs baseline: 1.5108x; 1.0126x over previous
"""Self-contained Trainium2 (Bass/Tile) DeformConv2d kernel.

kernel(x, offset, weight) -> np.ndarray [B, Cout, H, W] float32.
Data-parallel over batch: one SPMD Bass program per NeuronCore (8 cores).

Per core (one image): the bf16 x^T table lives in DRAM and is gathered with
4 int16 indices per (tap, pixel) — one per bilinear corner — laid out so the
4 corners of 32 pixels fill the 128 gather partitions (partition = 32*n + q).
DVE computes the 4 bilinear corner weights on-chip (L128 layout), 16 small
SBUF->SBUF DMAs shuffle them into the corner-stacked layout, and one narrow
[128,32] TensorScalarPtr per (tap, 32-px group) builds M = diag(w) @ K.
A single PE matmul per (tap, group, c-half) then performs
scale+combine+transpose+sum in one shot: valT[c,j] = sum_p g[p,c] * M[p,j].
The per-tap GEMM accumulates out[o, px] over taps in PSUM.
"""
import sys
import numpy as np

for _p in ("/opt/trn_rl_repo",):
    if _p not in sys.path:
        sys.path.insert(0, _p)

import concourse.bass as bass
import concourse.mybir as mybir
import concourse.tile as tile
from concourse import bacc
from concourse.bass_utils import run_bass_kernel_spmd

try:
    from ml_dtypes import bfloat16 as np_bf16
except ImportError:  # jax ships ml_dtypes; fall back via jax.numpy
    import jax.numpy as _jnp
    np_bf16 = _jnp.bfloat16

f32 = mybir.dt.float32
bf16 = mybir.dt.bfloat16
i32 = mybir.dt.int32
i16 = mybir.dt.int16
Alu = mybir.AluOpType
P = 128


def build_dcn(C=256, Cout=256, H=64, W=64, NT=9, NSTRIPE=8):
    HW = H * W
    S = HW // P              # 32 (L128 free index; s = pixel // 128)
    CB = C // P              # 2
    MB = Cout // P           # 2
    SPX = HW // NSTRIPE      # 512 pixels per stripe
    NSL = SPX // P           # 4 s_locals per stripe
    NG = SPX // 32           # 16 groups of 32 px per stripe
    NIDX = 4 * SPX           # 2048 gather slots per (tap, stripe)
    SW = NIDX // 16          # 128 idx free slots (16-wrap)
    FBIAS = 4.0 * max(H, W)
    FADD = FBIAS - 0.5       # HW f32->int cast is round-nearest-even

    nc = bacc.Bacc("TRN2", target_bir_lowering=False, debug=False)

    xtab = nc.declare_dram_parameter("xtab", [HW, C], bf16, isOutput=False)
    idxp = nc.declare_dram_parameter("idxp", [NSTRIPE, P, NT, SW], i16,
                                     isOutput=False)
    offy = nc.declare_dram_parameter("offy", [P, NT, S], f32, isOutput=False)
    offx = nc.declare_dram_parameter("offx", [P, NT, S], f32, isOutput=False)
    byc = nc.declare_dram_parameter("byc", [P, NT, S], f32, isOutput=False)
    bxc = nc.declare_dram_parameter("bxc", [P, NT, S], f32, isOutput=False)
    wt = nc.declare_dram_parameter("wt", [P, NT, CB, Cout], bf16, isOutput=False)
    kmat = nc.declare_dram_parameter("kmat", [P, 32], bf16, isOutput=False)
    out = nc.declare_dram_parameter("out", [Cout, HW], bf16, isOutput=True)

    xtab_ap = bass.AP(xtab[:].tensor, 0, [[C, HW], [1, C]])

    with tile.TileContext(nc) as tc:
        with tc.tile_pool(name="persist", bufs=1) as pp:
            idx_sb = pp.tile([P, NSTRIPE, NT, SW], i16, name="idx_sb")
            oy = pp.tile([P, NT, S], f32, name="oy")
            ox = pp.tile([P, NT, S], f32, name="ox")
            by = pp.tile([P, NT, S], f32, name="by")
            bx = pp.tile([P, NT, S], f32, name="bx")
            # offsets first (they gate the serial phase-1 chain), then
            # stripe-0 indices so gathers start immediately after
            nc.sync.dma_start(out=oy[:], in_=offy[:])
            nc.sync.dma_start(out=ox[:], in_=offx[:])
            nc.sync.dma_start(out=by[:], in_=byc[:])
            nc.sync.dma_start(out=bx[:], in_=bxc[:])
            nc.sync.dma_start(out=idx_sb[:, 0], in_=idxp[0])
            for st in range(1, NSTRIPE):
                nc.sync.dma_start(out=idx_sb[:, st], in_=idxp[st])
            wtb = pp.tile([P, NT, CB, Cout], bf16, name="wtb")
            nc.sync.dma_start(out=wtb[:], in_=wt[:])
            ksb = pp.tile([P, 32], bf16, name="ksb")
            nc.sync.dma_start(out=ksb[:], in_=kmat[:])
            # corner weights in corner-stacked layout: [p=32n+q, d, k, s]
            wcol = pp.tile([P, 4, NT, S], f32, name="wcol")

            with (
                tc.tile_pool(name="gather", bufs=10) as g_pool,
                tc.tile_pool(name="mtiles", bufs=4) as m_pool,
                tc.tile_pool(name="vout", bufs=3) as v_pool,
                tc.tile_pool(name="obuf", bufs=2) as o_pool,
                tc.tile_pool(name="psum_out", bufs=1, space="PSUM") as pso_pool,
                tc.tile_pool(name="psum_val", bufs=3, space="PSUM") as psv_pool,
            ):
                # ---- phase 1: bilinear corner weights (L128: px = 128s+p)
                with tc.tile_pool(name="ph1", bufs=1) as sp:
                    _ctr = [0]

                    def newt(nm=None):
                        if nm is None:
                            _ctr[0] += 1
                            nm = f"pt{_ctr[0]}"
                        return sp.tile([P, NT, S], f32, name=nm)

                    def tt(a, b, op):
                        o = newt()
                        nc.vector.tensor_tensor(out=o[:], in0=a[:], in1=b[:], op=op)
                        return o

                    def ts(a, s1, op0, s2=None, op1=None):
                        o = newt()
                        if s2 is None:
                            nc.vector.tensor_scalar(
                                out=o[:], in0=a[:], scalar1=float(s1),
                                scalar2=None, op0=op0,
                            )
                        else:
                            nc.vector.tensor_scalar(
                                out=o[:], in0=a[:], scalar1=float(s1),
                                scalar2=float(s2), op0=op0, op1=op1,
                            )
                        return o

                    py = tt(oy, by, Alu.add)
                    px = tt(ox, bx, Alu.add)

                    def floor_(v):
                        vb = ts(v, FADD, Alu.add)
                        vi = sp.tile([P, NT, S], i32, name=f"vi{_ctr[0]}")
                        nc.vector.tensor_copy(out=vi[:], in_=vb[:])
                        vf = newt()
                        nc.vector.tensor_copy(out=vf[:], in_=vi[:])
                        return ts(vf, FBIAS, Alu.subtract)

                    y0 = floor_(py)
                    x0 = floor_(px)
                    ly = tt(py, y0, Alu.subtract)
                    lx = tt(px, x0, Alu.subtract)

                    def rng_mask(v, lo, hi):
                        a = ts(v, lo, Alu.is_ge)
                        b = ts(v, hi, Alu.is_le)
                        return tt(a, b, Alu.mult)

                    my0 = rng_mask(y0, 0.0, H - 1)
                    my1 = rng_mask(y0, -1.0, H - 2)
                    mx0 = rng_mask(x0, 0.0, W - 1)
                    mx1 = rng_mask(x0, -1.0, W - 2)

                    one_m_ly = ts(ly, -1.0, Alu.mult, 1.0, Alu.add)
                    one_m_lx = ts(lx, -1.0, Alu.mult, 1.0, Alu.add)
                    vy0 = tt(one_m_ly, my0, Alu.mult)
                    vy1 = tt(ly, my1, Alu.mult)
                    ux0 = tt(one_m_lx, mx0, Alu.mult)
                    ux1 = tt(lx, mx1, Alu.mult)

                    w4 = [tt(vy0, ux0, Alu.mult), tt(vy0, ux1, Alu.mult),
                          tt(vy1, ux0, Alu.mult), tt(vy1, ux1, Alu.mult)]

                    # shuffle into corner-stacked layout via 16 tiny
                    # SBUF->SBUF DMAs: wcol[32n+q, d, k, s] = w_n[32d+q, k, s]
                    # (split across SP and Act queues to halve config time)
                    for n in range(4):
                        for d in range(4):
                            eng = nc.sync if (n % 2 == 0) else nc.scalar
                            eng.dma_start(
                                out=wcol[32 * n:32 * (n + 1), d],
                                in_=w4[n][32 * d:32 * (d + 1)],
                            )

                # ---- main loop
                for st in range(NSTRIPE):
                    out_ps = [
                        pso_pool.tile([P, SPX], f32, space="PSUM", name=f"ops{m}")
                        for m in range(MB)
                    ]
                    for k in range(NT):
                        gbuf = g_pool.tile([P, NG, C], bf16, name="gbuf")
                        nc.gpsimd.dma_gather(
                            gbuf[:], xtab_ap, idx_sb[:, st, k, :], NIDX, NIDX, C,
                        )
                        mall = m_pool.tile([P, NG, 32], bf16, name="mall")
                        for sl in range(NSL):
                            for d in range(4):
                                g = 4 * sl + d
                                s_g = NSL * st + sl
                                nc.vector.tensor_scalar(
                                    out=mall[:, g, :], in0=ksb[:],
                                    scalar1=wcol[:, d, k, s_g:s_g + 1],
                                    scalar2=None, op0=Alu.mult,
                                )
                        val_ps = [
                            psv_pool.tile([P, SPX], f32, space="PSUM",
                                          name=f"vps{cb}")
                            for cb in range(CB)
                        ]
                        for sl in range(NSL):
                            for d in range(4):
                                g = 4 * sl + d
                                col = sl * P + 32 * d
                                for cb in range(CB):
                                    nc.tensor.matmul(
                                        out=val_ps[cb][:, col:col + 32],
                                        lhsT=gbuf[:, g, cb * P:(cb + 1) * P],
                                        rhs=mall[:, g, :],
                                        start=True, stop=True,
                                    )
                        vsb = v_pool.tile([P, CB, SPX], bf16, name="vsb")
                        nc.vector.tensor_copy(out=vsb[:, 0], in_=val_ps[0][:])
                        nc.scalar.copy(out=vsb[:, 1], in_=val_ps[1][:])
                        for mb in range(MB):
                            for cb in range(CB):
                                nc.tensor.matmul(
                                    out=out_ps[mb][:],
                                    lhsT=wtb[:, k, cb, mb * P:(mb + 1) * P],
                                    rhs=vsb[:, cb],
                                    start=(k == 0 and cb == 0),
                                    stop=(k == NT - 1 and cb == CB - 1),
                                )
                    for mb in range(MB):
                        ob = o_pool.tile([P, SPX], bf16, name="ob")
                        nc.scalar.copy(out=ob[:], in_=out_ps[mb][:])
                        nc.sync.dma_start(
                            out=out[mb * P:(mb + 1) * P, st * SPX:(st + 1) * SPX],
                            in_=ob[:],
                        )

    nc.compile()
    return nc


def host_prep(x_b, offset_b, weight, H, W, KH, KW, PAD):
    """Per-core input map from one batch slice (numpy, f32)."""
    C = x_b.shape[0]
    Cout = weight.shape[0]
    HW = H * W
    S = HW // P
    NT = KH * KW
    CB = C // P
    NSTRIPE = 8
    SPX = HW // NSTRIPE
    NIDX = 4 * SPX
    SW = NIDX // 16

    xt = np.ascontiguousarray(x_b.reshape(C, HW).T).astype(np_bf16)
    off = offset_b.reshape(NT, 2, HW)
    j = np.arange(HW)
    ks = np.arange(NT)
    byv = (j[None, :] // W - PAD + (ks // KW)[:, None]).astype(np.float32)  # [k, j]
    bxv = (j[None, :] % W - PAD + (ks % KW)[:, None]).astype(np.float32)

    def l128(a):  # [k, j] -> [p, k, s], j = 128*s + p
        return np.ascontiguousarray(a.reshape(NT, S, P).transpose(2, 0, 1)).astype(np.float32)

    # per-corner clamped gather rows, mirroring the device f32 floor trick
    py = off[:, 0].astype(np.float32) + byv
    px = off[:, 1].astype(np.float32) + bxv
    FADD = np.float32(4.0 * max(H, W) - 0.5)
    y0 = np.rint(py + FADD).astype(np.int64) - int(4.0 * max(H, W))
    x0 = np.rint(px + FADD).astype(np.int64) - int(4.0 * max(H, W))
    rq = np.zeros((4, NT, HW), np.int64)
    for n in range(4):
        yn = np.clip(y0 + (n >> 1), 0, H - 1)
        xn = np.clip(x0 + (n & 1), 0, W - 1)
        rq[n] = yn * W + xn

    # slot order per (stripe, k): i = 128*(4*sl+d) + 32*n + q,
    # pixel = stripe*SPX + 128*sl + 32*d + q
    i_arr = np.arange(NIDX)
    sl_i = i_arr // 512
    d_i = (i_arr // 128) % 4
    n_i = (i_arr % 128) // 32
    q_i = i_arr % 32
    pxl = 128 * sl_i + 32 * d_i + q_i      # [NIDX]
    idx = np.zeros((NSTRIPE, P, NT, SW), np.int16)
    for st in range(NSTRIPE):
        rows = rq[n_i, :, st * SPX + pxl].astype(np.int16)  # [NIDX, NT]
        wrap = rows.reshape(SW, 16, NT).transpose(1, 2, 0)  # [16, NT, SW]
        idx[st] = np.tile(wrap, (8, 1, 1))                  # [128, NT, SW]

    wr = weight.reshape(Cout, C, NT)
    wtv = wr.reshape(Cout, CB, P, NT).transpose(2, 3, 1, 0)  # [p, k, cb, o]
    kmat = (np.arange(P)[:, None] % 32 == np.arange(32)[None, :])

    return {
        "xtab": xt,
        "idxp": idx,
        "offy": l128(off[:, 0]), "offx": l128(off[:, 1]),
        "byc": l128(byv), "bxc": l128(bxv),
        "wt": np.ascontiguousarray(wtv).astype(np_bf16),
        "kmat": kmat.astype(np_bf16),
    }


_NC_CACHE = {}


def _get_nc(key, **kw):
    if key not in _NC_CACHE:
        _NC_CACHE[key] = build_dcn(**kw)
    return _NC_CACHE[key]


def kernel(x, offset, weight):
    x = np.asarray(x, dtype=np.float32)
    offset = np.asarray(offset, dtype=np.float32)
    weight = np.asarray(weight, dtype=np.float32)
    B, C, H, W = x.shape
    Cout = weight.shape[0]
    KH, KW = weight.shape[2], weight.shape[3]
    PAD = 1
    assert B == 8 and C % 128 == 0 and Cout % 128 == 0
    nc = _get_nc((C, Cout, H, W, KH, KW), C=C, Cout=Cout, H=H, W=W,
                 NT=KH * KW)
    in_maps = [host_prep(x[b], offset[b], weight, H, W, KH, KW, PAD)
               for b in range(B)]
    res = run_bass_kernel_spmd(nc, in_maps, list(range(B)))
    out = np.stack([res.results[b]["out"].astype(np.float32).reshape(Cout, H, W)
                    for b in range(B)])
    return out


# revision 9
# speedup vs baseline: 1.5206x; 1.0065x over previous
"""Self-contained Trainium2 (Bass/Tile) DeformConv2d kernel.

kernel(x, offset, weight) -> np.ndarray [B, Cout, H, W] float32.
Data-parallel over batch: one SPMD Bass program per NeuronCore (8 cores).

Per core (one image): the bf16 x^T table lives in DRAM and is gathered with
4 int16 indices per (tap, pixel) — one per bilinear corner — laid out so the
4 corners of 32 pixels fill the 128 gather partitions (partition = 32*n + q).
DVE computes the 4 bilinear corner weights on-chip (L128 layout), 16 small
SBUF->SBUF DMAs shuffle them into the corner-stacked layout, and one narrow
[128,32] TensorScalarPtr per (tap, 32-px group) builds M = diag(w) @ K.
A single PE matmul per (tap, group, c-half) then performs
scale+combine+transpose+sum in one shot: valT[c,j] = sum_p g[p,c] * M[p,j].
The per-tap GEMM accumulates out[o, px] over taps in PSUM.
"""
import sys
import numpy as np

for _p in ("/opt/trn_rl_repo",):
    if _p not in sys.path:
        sys.path.insert(0, _p)

import concourse.bass as bass
import concourse.mybir as mybir
import concourse.tile as tile
from concourse import bacc
from concourse.bass_utils import run_bass_kernel_spmd

try:
    from ml_dtypes import bfloat16 as np_bf16
except ImportError:  # jax ships ml_dtypes; fall back via jax.numpy
    import jax.numpy as _jnp
    np_bf16 = _jnp.bfloat16

f32 = mybir.dt.float32
bf16 = mybir.dt.bfloat16
i32 = mybir.dt.int32
i16 = mybir.dt.int16
Alu = mybir.AluOpType
P = 128


def build_dcn(C=256, Cout=256, H=64, W=64, NT=9, NSTRIPE=8):
    HW = H * W
    S = HW // P              # 32 (L128 free index; s = pixel // 128)
    CB = C // P              # 2
    MB = Cout // P           # 2
    SPX = HW // NSTRIPE      # 512 pixels per stripe
    NSL = SPX // P           # 4 s_locals per stripe
    NG = SPX // 32           # 16 groups of 32 px per stripe
    NIDX = 4 * SPX           # 2048 gather slots per (tap, stripe)
    SW = NIDX // 16          # 128 idx free slots (16-wrap)
    FBIAS = 4.0 * max(H, W)
    FADD = FBIAS - 0.5       # HW f32->int cast is round-nearest-even

    nc = bacc.Bacc("TRN2", target_bir_lowering=False, debug=False)

    xtab = nc.declare_dram_parameter("xtab", [HW, C], bf16, isOutput=False)
    idxp = nc.declare_dram_parameter("idxp", [NSTRIPE, P, NT, SW], i16,
                                     isOutput=False)
    offy = nc.declare_dram_parameter("offy", [P, NT, S], f32, isOutput=False)
    offx = nc.declare_dram_parameter("offx", [P, NT, S], f32, isOutput=False)
    byc = nc.declare_dram_parameter("byc", [P, NT, S], f32, isOutput=False)
    bxc = nc.declare_dram_parameter("bxc", [P, NT, S], f32, isOutput=False)
    wt = nc.declare_dram_parameter("wt", [P, NT, CB, Cout], bf16, isOutput=False)
    kmat = nc.declare_dram_parameter("kmat", [P, 32], bf16, isOutput=False)
    out = nc.declare_dram_parameter("out", [Cout, HW], bf16, isOutput=True)

    xtab_ap = bass.AP(xtab[:].tensor, 0, [[C, HW], [1, C]])

    with tile.TileContext(nc) as tc:
        with tc.tile_pool(name="persist", bufs=1) as pp:
            idx_sb = pp.tile([P, NSTRIPE, NT, SW], i16, name="idx_sb")
            oy = pp.tile([P, NT, S], f32, name="oy")
            ox = pp.tile([P, NT, S], f32, name="ox")
            by = pp.tile([P, NT, S], f32, name="by")
            bx = pp.tile([P, NT, S], f32, name="bx")
            # offsets first (they gate the serial phase-1 chain), then
            # stripe-0 indices so gathers start immediately after
            nc.sync.dma_start(out=oy[:], in_=offy[:])
            nc.scalar.dma_start(out=ox[:], in_=offx[:])
            nc.sync.dma_start(out=by[:], in_=byc[:])
            nc.scalar.dma_start(out=bx[:], in_=bxc[:])
            nc.sync.dma_start(out=idx_sb[:, 0], in_=idxp[0])
            for st in range(1, NSTRIPE):
                eng = nc.sync if st % 2 else nc.scalar
                eng.dma_start(out=idx_sb[:, st], in_=idxp[st])
            wtb = pp.tile([P, NT, CB, Cout], bf16, name="wtb")
            nc.scalar.dma_start(out=wtb[:], in_=wt[:])
            ksb = pp.tile([P, 32], bf16, name="ksb")
            nc.sync.dma_start(out=ksb[:], in_=kmat[:])
            # corner weights in corner-stacked layout: [p=32n+q, d, k, s]
            wcol = pp.tile([P, 4, NT, S], f32, name="wcol")

            with (
                tc.tile_pool(name="gather", bufs=12) as g_pool,
                tc.tile_pool(name="mtiles", bufs=8) as m_pool,
                tc.tile_pool(name="vout", bufs=3) as v_pool,
                tc.tile_pool(name="obuf", bufs=2) as o_pool,
                tc.tile_pool(name="psum_out", bufs=1, space="PSUM") as pso_pool,
                tc.tile_pool(name="psum_val", bufs=3, space="PSUM") as psv_pool,
            ):
                # ---- phase 1: bilinear corner weights (L128: px = 128s+p)
                with tc.tile_pool(name="ph1", bufs=1) as sp:
                    names = ["py", "px", "y0", "x0", "ly", "lx",
                             "my0", "my1", "mx0", "mx1",
                             "vy0", "vy1", "ux0", "ux1",
                             "w0", "w1", "w2", "w3", "sa", "sb"]
                    T = {nm: sp.tile([P, NT, S], f32, name=nm) for nm in names}
                    vi = sp.tile([P, NT, S], i32, name="vi")

                    def tt(o, a, b, op):
                        nc.vector.tensor_tensor(out=T[o][:], in0=T[a][:],
                                                in1=T[b][:], op=op)

                    def ts(o, a, s1, op0, s2=None, op1=None):
                        if s2 is None:
                            nc.vector.tensor_scalar(
                                out=T[o][:], in0=T[a][:], scalar1=float(s1),
                                scalar2=None, op0=op0,
                            )
                        else:
                            nc.vector.tensor_scalar(
                                out=T[o][:], in0=T[a][:], scalar1=float(s1),
                                scalar2=float(s2), op0=op0, op1=op1,
                            )

                    nc.vector.tensor_tensor(out=T["py"][:], in0=oy[:],
                                            in1=by[:], op=Alu.add)
                    nc.vector.tensor_tensor(out=T["px"][:], in0=ox[:],
                                            in1=bx[:], op=Alu.add)

                    def floor_(dst, src):
                        ts("sa", src, FADD, Alu.add)
                        nc.vector.tensor_copy(out=vi[:], in_=T["sa"][:])
                        nc.vector.tensor_copy(out=T["sb"][:], in_=vi[:])
                        ts(dst, "sb", FBIAS, Alu.subtract)

                    def rng_mask(dst, v, lo, hi):
                        # mask = (clamp(v, lo, hi) == v)
                        ts("sa", v, lo, Alu.max, hi, Alu.min)
                        tt(dst, "sa", v, Alu.is_equal)

                    floor_("y0", "py")
                    tt("ly", "py", "y0", Alu.subtract)
                    rng_mask("my0", "y0", 0.0, H - 1)
                    rng_mask("my1", "y0", -1.0, H - 2)
                    floor_("x0", "px")
                    tt("lx", "px", "x0", Alu.subtract)
                    rng_mask("mx0", "x0", 0.0, W - 1)
                    rng_mask("mx1", "x0", -1.0, W - 2)

                    ts("sa", "ly", -1.0, Alu.mult, 1.0, Alu.add)
                    tt("vy0", "sa", "my0", Alu.mult)
                    tt("vy1", "ly", "my1", Alu.mult)
                    ts("sb", "lx", -1.0, Alu.mult, 1.0, Alu.add)
                    tt("ux0", "sb", "mx0", Alu.mult)
                    tt("ux1", "lx", "mx1", Alu.mult)

                    tt("w0", "vy0", "ux0", Alu.mult)
                    tt("w1", "vy0", "ux1", Alu.mult)
                    tt("w2", "vy1", "ux0", Alu.mult)
                    tt("w3", "vy1", "ux1", Alu.mult)
                    w4 = [T["w0"], T["w1"], T["w2"], T["w3"]]

                    # shuffle into corner-stacked layout via 16 tiny
                    # SBUF->SBUF DMAs: wcol[32n+q, d, k, s] = w_n[32d+q, k, s]
                    # (split across SP and Act queues to halve config time)
                    for n in range(4):
                        for d in range(4):
                            eng = nc.sync if (n % 2 == 0) else nc.scalar
                            eng.dma_start(
                                out=wcol[32 * n:32 * (n + 1), d],
                                in_=w4[n][32 * d:32 * (d + 1)],
                            )

                # ---- main loop
                for st in range(NSTRIPE):
                    out_ps = [
                        pso_pool.tile([P, SPX], f32, space="PSUM", name=f"ops{m}")
                        for m in range(MB)
                    ]
                    for k in range(NT):
                        gbuf = g_pool.tile([P, NG, C], bf16, name="gbuf")
                        nc.gpsimd.dma_gather(
                            gbuf[:], xtab_ap, idx_sb[:, st, k, :], NIDX, NIDX, C,
                        )
                        mall = m_pool.tile([P, NG, 32], bf16, name="mall")
                        for sl in range(NSL):
                            for d in range(4):
                                g = 4 * sl + d
                                s_g = NSL * st + sl
                                nc.vector.tensor_scalar(
                                    out=mall[:, g, :], in0=ksb[:],
                                    scalar1=wcol[:, d, k, s_g:s_g + 1],
                                    scalar2=None, op0=Alu.mult,
                                )
                        val_ps = [
                            psv_pool.tile([P, SPX], f32, space="PSUM",
                                          name=f"vps{cb}")
                            for cb in range(CB)
                        ]
                        for sl in range(NSL):
                            for d in range(4):
                                g = 4 * sl + d
                                col = sl * P + 32 * d
                                for cb in range(CB):
                                    nc.tensor.matmul(
                                        out=val_ps[cb][:, col:col + 32],
                                        lhsT=gbuf[:, g, cb * P:(cb + 1) * P],
                                        rhs=mall[:, g, :],
                                        start=True, stop=True,
                                    )
                        vsb = v_pool.tile([P, CB, SPX], bf16, name="vsb")
                        nc.vector.tensor_copy(out=vsb[:, 0], in_=val_ps[0][:])
                        nc.scalar.copy(out=vsb[:, 1], in_=val_ps[1][:])
                        for mb in range(MB):
                            for cb in range(CB):
                                nc.tensor.matmul(
                                    out=out_ps[mb][:],
                                    lhsT=wtb[:, k, cb, mb * P:(mb + 1) * P],
                                    rhs=vsb[:, cb],
                                    start=(k == 0 and cb == 0),
                                    stop=(k == NT - 1 and cb == CB - 1),
                                )
                    for mb in range(MB):
                        ob = o_pool.tile([P, SPX], bf16, name="ob")
                        nc.scalar.copy(out=ob[:], in_=out_ps[mb][:])
                        nc.sync.dma_start(
                            out=out[mb * P:(mb + 1) * P, st * SPX:(st + 1) * SPX],
                            in_=ob[:],
                        )

    nc.compile()
    return nc


def host_prep(x_b, offset_b, weight, H, W, KH, KW, PAD):
    """Per-core input map from one batch slice (numpy, f32)."""
    C = x_b.shape[0]
    Cout = weight.shape[0]
    HW = H * W
    S = HW // P
    NT = KH * KW
    CB = C // P
    NSTRIPE = 8
    SPX = HW // NSTRIPE
    NIDX = 4 * SPX
    SW = NIDX // 16

    xt = np.ascontiguousarray(x_b.reshape(C, HW).T).astype(np_bf16)
    off = offset_b.reshape(NT, 2, HW)
    j = np.arange(HW)
    ks = np.arange(NT)
    byv = (j[None, :] // W - PAD + (ks // KW)[:, None]).astype(np.float32)  # [k, j]
    bxv = (j[None, :] % W - PAD + (ks % KW)[:, None]).astype(np.float32)

    def l128(a):  # [k, j] -> [p, k, s], j = 128*s + p
        return np.ascontiguousarray(a.reshape(NT, S, P).transpose(2, 0, 1)).astype(np.float32)

    # per-corner clamped gather rows, mirroring the device f32 floor trick
    py = off[:, 0].astype(np.float32) + byv
    px = off[:, 1].astype(np.float32) + bxv
    FADD = np.float32(4.0 * max(H, W) - 0.5)
    y0 = np.rint(py + FADD).astype(np.int64) - int(4.0 * max(H, W))
    x0 = np.rint(px + FADD).astype(np.int64) - int(4.0 * max(H, W))
    rq = np.zeros((4, NT, HW), np.int64)
    for n in range(4):
        yn = np.clip(y0 + (n >> 1), 0, H - 1)
        xn = np.clip(x0 + (n & 1), 0, W - 1)
        rq[n] = yn * W + xn

    # slot order per (stripe, k): i = 128*(4*sl+d) + 32*n + q,
    # pixel = stripe*SPX + 128*sl + 32*d + q
    i_arr = np.arange(NIDX)
    sl_i = i_arr // 512
    d_i = (i_arr // 128) % 4
    n_i = (i_arr % 128) // 32
    q_i = i_arr % 32
    pxl = 128 * sl_i + 32 * d_i + q_i      # [NIDX]
    idx = np.zeros((NSTRIPE, P, NT, SW), np.int16)
    for st in range(NSTRIPE):
        rows = rq[n_i, :, st * SPX + pxl].astype(np.int16)  # [NIDX, NT]
        wrap = rows.reshape(SW, 16, NT).transpose(1, 2, 0)  # [16, NT, SW]
        idx[st] = np.tile(wrap, (8, 1, 1))                  # [128, NT, SW]

    wr = weight.reshape(Cout, C, NT)
    wtv = wr.reshape(Cout, CB, P, NT).transpose(2, 3, 1, 0)  # [p, k, cb, o]
    kmat = (np.arange(P)[:, None] % 32 == np.arange(32)[None, :])

    return {
        "xtab": xt,
        "idxp": idx,
        "offy": l128(off[:, 0]), "offx": l128(off[:, 1]),
        "byc": l128(byv), "bxc": l128(bxv),
        "wt": np.ascontiguousarray(wtv).astype(np_bf16),
        "kmat": kmat.astype(np_bf16),
    }


_NC_CACHE = {}


def _get_nc(key, **kw):
    if key not in _NC_CACHE:
        _NC_CACHE[key] = build_dcn(**kw)
    return _NC_CACHE[key]


def kernel(x, offset, weight):
    x = np.asarray(x, dtype=np.float32)
    offset = np.asarray(offset, dtype=np.float32)
    weight = np.asarray(weight, dtype=np.float32)
    B, C, H, W = x.shape
    Cout = weight.shape[0]
    KH, KW = weight.shape[2], weight.shape[3]
    PAD = 1
    assert B == 8 and C % 128 == 0 and Cout % 128 == 0
    nc = _get_nc((C, Cout, H, W, KH, KW), C=C, Cout=Cout, H=H, W=W,
                 NT=KH * KW)
    in_maps = [host_prep(x[b], offset[b], weight, H, W, KH, KW, PAD)
               for b in range(B)]
    res = run_bass_kernel_spmd(nc, in_maps, list(range(B)))
    out = np.stack([res.results[b]["out"].astype(np.float32).reshape(Cout, H, W)
                    for b in range(B)])
    return out


# revision 11
# speedup vs baseline: 1.5510x; 1.0200x over previous
"""Self-contained Trainium2 (Bass/Tile) DeformConv2d kernel.

kernel(x, offset, weight) -> np.ndarray [B, Cout, H, W] float32.
Data-parallel over batch: one SPMD Bass program per NeuronCore (8 cores).

Per core (one image): the bf16 x^T table lives in DRAM and is gathered with
4 int16 indices per (tap, pixel) — one per bilinear corner — laid out so the
4 corners of 32 pixels fill the 128 gather partitions (partition = 32*n + q).
DVE computes the 4 bilinear corner weights on-chip (L128 layout), 16 small
SBUF->SBUF DMAs shuffle them into the corner-stacked layout, and one narrow
[128,32] TensorScalarPtr per (tap, 32-px group) builds M = diag(w) @ K.
A single PE matmul per (tap, group, c-half) then performs
scale+combine+transpose+sum in one shot: valT[c,j] = sum_p g[p,c] * M[p,j].
The per-tap GEMM accumulates out[o, px] over taps in PSUM.
"""
import sys
import numpy as np

for _p in ("/opt/trn_rl_repo",):
    if _p not in sys.path:
        sys.path.insert(0, _p)

import concourse.bass as bass
import concourse.mybir as mybir
import concourse.tile as tile
from concourse import bacc
from concourse.bass_utils import run_bass_kernel_spmd

try:
    from ml_dtypes import bfloat16 as np_bf16
except ImportError:  # jax ships ml_dtypes; fall back via jax.numpy
    import jax.numpy as _jnp
    np_bf16 = _jnp.bfloat16

f32 = mybir.dt.float32
bf16 = mybir.dt.bfloat16
i32 = mybir.dt.int32
i16 = mybir.dt.int16
Alu = mybir.AluOpType
P = 128


def build_dcn(C=256, Cout=256, H=64, W=64, NT=9, NSTRIPE=8):
    HW = H * W
    S = HW // P              # 32 (L128 free index; s = pixel // 128)
    CB = C // P              # 2
    MB = Cout // P           # 2
    SPX = HW // NSTRIPE      # 512 pixels per stripe
    NSL = SPX // P           # 4 s_locals per stripe
    NG = SPX // 32           # 16 groups of 32 px per stripe
    NIDX = 4 * SPX           # 2048 gather slots per (tap, stripe)
    SW = NIDX // 16          # 128 idx free slots (16-wrap)
    FBIAS = 4.0 * max(H, W)
    FADD = FBIAS - 0.5       # HW f32->int cast is round-nearest-even

    nc = bacc.Bacc("TRN2", target_bir_lowering=False, debug=False)

    xtab = nc.declare_dram_parameter("xtab", [HW, C], bf16, isOutput=False)
    idxp = nc.declare_dram_parameter("idxp", [NSTRIPE, P, NT, SW], i16,
                                     isOutput=False)
    offy = nc.declare_dram_parameter("offy", [P, NT, S], f32, isOutput=False)
    offx = nc.declare_dram_parameter("offx", [P, NT, S], f32, isOutput=False)
    byc = nc.declare_dram_parameter("byc", [P, NT, S], f32, isOutput=False)
    bxc = nc.declare_dram_parameter("bxc", [P, NT, S], f32, isOutput=False)
    wt = nc.declare_dram_parameter("wt", [P, NT, CB, Cout], bf16, isOutput=False)
    kmat = nc.declare_dram_parameter("kmat", [P, 32], bf16, isOutput=False)
    perm = nc.declare_dram_parameter("perm", [P, 4, 4, P], bf16, isOutput=False)
    out = nc.declare_dram_parameter("out", [Cout, HW], bf16, isOutput=True)

    xtab_ap = bass.AP(xtab[:].tensor, 0, [[C, HW], [1, C]])

    with tile.TileContext(nc) as tc:
        with tc.tile_pool(name="persist", bufs=1) as pp:
            idx_sb = pp.tile([P, NSTRIPE, NT, SW], i16, name="idx_sb")
            oy = pp.tile([P, NT, S], f32, name="oy")
            ox = pp.tile([P, NT, S], f32, name="ox")
            by = pp.tile([P, NT, S], f32, name="by")
            bx = pp.tile([P, NT, S], f32, name="bx")
            # offsets first (they gate the serial phase-1 chain), then
            # stripe-0 indices so gathers start immediately after
            nc.sync.dma_start(out=oy[:], in_=offy[:])
            nc.sync.dma_start(out=ox[:], in_=offx[:])
            nc.sync.dma_start(out=by[:], in_=byc[:])
            nc.sync.dma_start(out=bx[:], in_=bxc[:])
            nc.sync.dma_start(out=idx_sb[:, 0], in_=idxp[0])
            perm_sb = pp.tile([P, 4, 4, P], bf16, name="perm_sb")
            nc.sync.dma_start(out=perm_sb[:], in_=perm[:])
            for st in range(1, NSTRIPE):
                nc.sync.dma_start(out=idx_sb[:, st], in_=idxp[st])
            wtb = pp.tile([P, NT, CB, Cout], bf16, name="wtb")
            nc.sync.dma_start(out=wtb[:], in_=wt[:])
            ksb = pp.tile([P, 32], bf16, name="ksb")
            nc.sync.dma_start(out=ksb[:], in_=kmat[:])
            # corner weights in corner-stacked layout: [p=32n+q, d, k, s]
            wcol = pp.tile([P, 4, NT, S], f32, name="wcol")

            with (
                tc.tile_pool(name="gather", bufs=12) as g_pool,
                tc.tile_pool(name="mtiles", bufs=8) as m_pool,
                tc.tile_pool(name="vout", bufs=3) as v_pool,
                tc.tile_pool(name="obuf", bufs=2) as o_pool,
            ):
                # ---- phase 1: bilinear corner weights (L128: px = 128s+p)
                with tc.tile_pool(name="ph1", bufs=1) as sp, \
                     tc.tile_pool(name="ph1ps", bufs=1, space="PSUM") as spp:
                    names = ["py", "px", "y0", "x0", "ly", "lx",
                             "my0", "my1", "mx0", "mx1",
                             "vy0", "vy1", "ux0", "ux1",
                             "w0", "w1", "w2", "w3", "sa", "sb"]
                    T = {nm: sp.tile([P, NT, S], f32, name=nm) for nm in names}
                    vi = sp.tile([P, NT, S], i32, name="vi")

                    def tt(o, a, b, op):
                        nc.vector.tensor_tensor(out=T[o][:], in0=T[a][:],
                                                in1=T[b][:], op=op)

                    def ts(o, a, s1, op0, s2=None, op1=None):
                        if s2 is None:
                            nc.vector.tensor_scalar(
                                out=T[o][:], in0=T[a][:], scalar1=float(s1),
                                scalar2=None, op0=op0,
                            )
                        else:
                            nc.vector.tensor_scalar(
                                out=T[o][:], in0=T[a][:], scalar1=float(s1),
                                scalar2=float(s2), op0=op0, op1=op1,
                            )

                    nc.vector.tensor_tensor(out=T["py"][:], in0=oy[:],
                                            in1=by[:], op=Alu.add)
                    nc.vector.tensor_tensor(out=T["px"][:], in0=ox[:],
                                            in1=bx[:], op=Alu.add)

                    def floor_(dst, src):
                        ts("sa", src, FADD, Alu.add)
                        nc.vector.tensor_copy(out=vi[:], in_=T["sa"][:])
                        nc.vector.tensor_copy(out=T["sb"][:], in_=vi[:])
                        ts(dst, "sb", FBIAS, Alu.subtract)

                    def rng_mask(dst, v, lo, hi):
                        # mask = (clamp(v, lo, hi) == v)
                        ts("sa", v, lo, Alu.max, hi, Alu.min)
                        tt(dst, "sa", v, Alu.is_equal)

                    floor_("y0", "py")
                    tt("ly", "py", "y0", Alu.subtract)
                    rng_mask("my0", "y0", 0.0, H - 1)
                    rng_mask("my1", "y0", -1.0, H - 2)
                    floor_("x0", "px")
                    tt("lx", "px", "x0", Alu.subtract)
                    rng_mask("mx0", "x0", 0.0, W - 1)
                    rng_mask("mx1", "x0", -1.0, W - 2)

                    ts("sa", "ly", -1.0, Alu.mult, 1.0, Alu.add)
                    tt("vy0", "sa", "my0", Alu.mult)
                    tt("vy1", "ly", "my1", Alu.mult)
                    ts("sb", "lx", -1.0, Alu.mult, 1.0, Alu.add)
                    tt("ux0", "sb", "mx0", Alu.mult)
                    tt("ux1", "lx", "mx1", Alu.mult)

                    tt("w0", "vy0", "ux0", Alu.mult)
                    tt("w1", "vy0", "ux1", Alu.mult)
                    tt("w2", "vy1", "ux0", Alu.mult)
                    tt("w3", "vy1", "ux1", Alu.mult)
                    w4 = [T["w0"], T["w1"], T["w2"], T["w3"]]

                    # shuffle into corner-stacked layout on the PE:
                    # wcol[32n+q, d, k, s] = w_n[32d+q, k, s] via 16 constant
                    # permutation matmuls (PSUM-accumulated over n per d)
                    w4b = sp.tile([P, 4, NT, S], bf16, name="w4b")
                    for n in range(4):
                        nc.vector.tensor_copy(out=w4b[:, n], in_=w4[n][:])
                    wps = spp.tile([P, 4, NT * S], f32, space="PSUM", name="wps")
                    for d in range(4):
                        for n in range(4):
                            nc.tensor.matmul(
                                out=wps[:, d],
                                lhsT=perm_sb[:, n, d, :],
                                rhs=w4b[:, n].rearrange("p k s -> p (k s)"),
                                start=(n == 0), stop=(n == 3),
                            )
                    nc.scalar.copy(
                        out=wcol[:].rearrange("p d k s -> p (d k s)"), in_=wps[:])

                # ---- main loop
                with (
                    tc.tile_pool(name="psum_out", bufs=1, space="PSUM") as pso_pool,
                    tc.tile_pool(name="psum_val", bufs=3, space="PSUM") as psv_pool,
                ):
                  for st in range(NSTRIPE):
                    out_ps = [
                        pso_pool.tile([P, SPX], f32, space="PSUM", name=f"ops{m}")
                        for m in range(MB)
                    ]
                    for k in range(NT):
                        gbuf = g_pool.tile([P, NG, C], bf16, name="gbuf")
                        nc.gpsimd.dma_gather(
                            gbuf[:], xtab_ap, idx_sb[:, st, k, :], NIDX, NIDX, C,
                        )
                        mall = m_pool.tile([P, NG, 32], bf16, name="mall")
                        for sl in range(NSL):
                            for d in range(4):
                                g = 4 * sl + d
                                s_g = NSL * st + sl
                                nc.vector.tensor_scalar(
                                    out=mall[:, g, :], in0=ksb[:],
                                    scalar1=wcol[:, d, k, s_g:s_g + 1],
                                    scalar2=None, op0=Alu.mult,
                                )
                        val_ps = [
                            psv_pool.tile([P, SPX], f32, space="PSUM",
                                          name=f"vps{cb}")
                            for cb in range(CB)
                        ]
                        for sl in range(NSL):
                            for d in range(4):
                                g = 4 * sl + d
                                col = sl * P + 32 * d
                                for cb in range(CB):
                                    nc.tensor.matmul(
                                        out=val_ps[cb][:, col:col + 32],
                                        lhsT=gbuf[:, g, cb * P:(cb + 1) * P],
                                        rhs=mall[:, g, :],
                                        start=True, stop=True,
                                    )
                        vsb = v_pool.tile([P, CB, SPX], bf16, name="vsb")
                        nc.vector.tensor_copy(out=vsb[:, 0], in_=val_ps[0][:])
                        nc.scalar.copy(out=vsb[:, 1], in_=val_ps[1][:])
                        for mb in range(MB):
                            for cb in range(CB):
                                nc.tensor.matmul(
                                    out=out_ps[mb][:],
                                    lhsT=wtb[:, k, cb, mb * P:(mb + 1) * P],
                                    rhs=vsb[:, cb],
                                    start=(k == 0 and cb == 0),
                                    stop=(k == NT - 1 and cb == CB - 1),
                                )
                    for mb in range(MB):
                        ob = o_pool.tile([P, SPX], bf16, name="ob")
                        nc.scalar.copy(out=ob[:], in_=out_ps[mb][:])
                        nc.sync.dma_start(
                            out=out[mb * P:(mb + 1) * P, st * SPX:(st + 1) * SPX],
                            in_=ob[:],
                        )

    nc.compile()
    return nc


def host_prep(x_b, offset_b, weight, H, W, KH, KW, PAD):
    """Per-core input map from one batch slice (numpy, f32)."""
    C = x_b.shape[0]
    Cout = weight.shape[0]
    HW = H * W
    S = HW // P
    NT = KH * KW
    CB = C // P
    NSTRIPE = 8
    SPX = HW // NSTRIPE
    NIDX = 4 * SPX
    SW = NIDX // 16

    xt = np.ascontiguousarray(x_b.reshape(C, HW).T).astype(np_bf16)
    off = offset_b.reshape(NT, 2, HW)
    j = np.arange(HW)
    ks = np.arange(NT)
    byv = (j[None, :] // W - PAD + (ks // KW)[:, None]).astype(np.float32)  # [k, j]
    bxv = (j[None, :] % W - PAD + (ks % KW)[:, None]).astype(np.float32)

    def l128(a):  # [k, j] -> [p, k, s], j = 128*s + p
        return np.ascontiguousarray(a.reshape(NT, S, P).transpose(2, 0, 1)).astype(np.float32)

    # per-corner clamped gather rows, mirroring the device f32 floor trick
    py = off[:, 0].astype(np.float32) + byv
    px = off[:, 1].astype(np.float32) + bxv
    FADD = np.float32(4.0 * max(H, W) - 0.5)
    y0 = np.rint(py + FADD).astype(np.int64) - int(4.0 * max(H, W))
    x0 = np.rint(px + FADD).astype(np.int64) - int(4.0 * max(H, W))
    rq = np.zeros((4, NT, HW), np.int64)
    for n in range(4):
        yn = np.clip(y0 + (n >> 1), 0, H - 1)
        xn = np.clip(x0 + (n & 1), 0, W - 1)
        rq[n] = yn * W + xn

    # slot order per (stripe, k): i = 128*(4*sl+d) + 32*n + q,
    # pixel = stripe*SPX + 128*sl + 32*d + q
    i_arr = np.arange(NIDX)
    sl_i = i_arr // 512
    d_i = (i_arr // 128) % 4
    n_i = (i_arr % 128) // 32
    q_i = i_arr % 32
    pxl = 128 * sl_i + 32 * d_i + q_i      # [NIDX]
    idx = np.zeros((NSTRIPE, P, NT, SW), np.int16)
    for st in range(NSTRIPE):
        rows = rq[n_i, :, st * SPX + pxl].astype(np.int16)  # [NIDX, NT]
        wrap = rows.reshape(SW, 16, NT).transpose(1, 2, 0)  # [16, NT, SW]
        idx[st] = np.tile(wrap, (8, 1, 1))                  # [128, NT, SW]

    wr = weight.reshape(Cout, C, NT)
    wtv = wr.reshape(Cout, CB, P, NT).transpose(2, 3, 1, 0)  # [p, k, cb, o]
    kmat = (np.arange(P)[:, None] % 32 == np.arange(32)[None, :])
    # permutation constants: E[n,d][p,i] = 1 iff p == 32d+q and i == 32n+q
    E = np.zeros((4, 4, P, P), np.float32)
    q = np.arange(32)
    for n in range(4):
        for d in range(4):
            E[n, d, 32 * d + q, 32 * n + q] = 1.0
    permv = np.ascontiguousarray(E.transpose(2, 0, 1, 3))  # [p, n, d, i]

    return {
        "xtab": xt,
        "idxp": idx,
        "offy": l128(off[:, 0]), "offx": l128(off[:, 1]),
        "byc": l128(byv), "bxc": l128(bxv),
        "wt": np.ascontiguousarray(wtv).astype(np_bf16),
        "kmat": kmat.astype(np_bf16),
        "perm": permv.astype(np_bf16),
    }


_NC_CACHE = {}


def _get_nc(key, **kw):
    if key not in _NC_CACHE:
        _NC_CACHE[key] = build_dcn(**kw)
    return _NC_CACHE[key]


def kernel(x, offset, weight):
    x = np.asarray(x, dtype=np.float32)
    offset = np.asarray(offset, dtype=np.float32)
    weight = np.asarray(weight, dtype=np.float32)
    B, C, H, W = x.shape
    Cout = weight.shape[0]
    KH, KW = weight.shape[2], weight.shape[3]
    PAD = 1
    assert B == 8 and C % 128 == 0 and Cout % 128 == 0
    nc = _get_nc((C, Cout, H, W, KH, KW), C=C, Cout=Cout, H=H, W=W,
                 NT=KH * KW)
    in_maps = [host_prep(x[b], offset[b], weight, H, W, KH, KW, PAD)
               for b in range(B)]
    res = run_bass_kernel_spmd(nc, in_maps, list(range(B)))
    out = np.stack([res.results[b]["out"].astype(np.float32).reshape(Cout, H, W)
                    for b in range(B)])
    return out


# revision 14
# speedup vs baseline: 1.5528x; 1.0012x over previous
"""Self-contained Trainium2 (Bass/Tile) DeformConv2d kernel.

kernel(x, offset, weight) -> np.ndarray [B, Cout, H, W] float32.
Data-parallel over batch: one SPMD Bass program per NeuronCore (8 cores).

Per core (one image): the bf16 x^T table lives in DRAM and is gathered with
4 int16 indices per (tap, pixel) — one per bilinear corner — laid out so the
4 corners of 32 pixels fill the 128 gather partitions (partition = 32*n + q).
DVE computes the 4 bilinear corner weights on-chip (L128 layout), 16 small
SBUF->SBUF DMAs shuffle them into the corner-stacked layout, and one narrow
[128,32] TensorScalarPtr per (tap, 32-px group) builds M = diag(w) @ K.
A single PE matmul per (tap, group, c-half) then performs
scale+combine+transpose+sum in one shot: valT[c,j] = sum_p g[p,c] * M[p,j].
The per-tap GEMM accumulates out[o, px] over taps in PSUM.
"""
import sys
import numpy as np

for _p in ("/opt/trn_rl_repo",):
    if _p not in sys.path:
        sys.path.insert(0, _p)

import concourse.bass as bass
import concourse.mybir as mybir
import concourse.tile as tile
from concourse import bacc
from concourse.bass_utils import run_bass_kernel_spmd

try:
    from ml_dtypes import bfloat16 as np_bf16
except ImportError:  # jax ships ml_dtypes; fall back via jax.numpy
    import jax.numpy as _jnp
    np_bf16 = _jnp.bfloat16

f32 = mybir.dt.float32
bf16 = mybir.dt.bfloat16
i32 = mybir.dt.int32
i16 = mybir.dt.int16
Alu = mybir.AluOpType
P = 128


def build_dcn(C=256, Cout=256, H=64, W=64, NT=9, NSTRIPE=8):
    HW = H * W
    S = HW // P              # 32 (L128 free index; s = pixel // 128)
    CB = C // P              # 2
    MB = Cout // P           # 2
    SPX = HW // NSTRIPE      # 512 pixels per stripe
    NSL = SPX // P           # 4 s_locals per stripe
    NG = SPX // 32           # 16 groups of 32 px per stripe
    NIDX = 4 * SPX           # 2048 gather slots per (tap, stripe)
    SW = NIDX // 16          # 128 idx free slots (16-wrap)
    FBIAS = 4.0 * max(H, W)
    FADD = FBIAS - 0.5       # HW f32->int cast is round-nearest-even

    nc = bacc.Bacc("TRN2", target_bir_lowering=False, debug=False)

    xtab = nc.declare_dram_parameter("xtab", [HW, C], bf16, isOutput=False)
    idxp = nc.declare_dram_parameter("idxp", [NSTRIPE, P, NT, SW], i16,
                                     isOutput=False)
    offy = nc.declare_dram_parameter("offy", [P, NT, S], f32, isOutput=False)
    offx = nc.declare_dram_parameter("offx", [P, NT, S], f32, isOutput=False)
    byc = nc.declare_dram_parameter("byc", [P, NT, S], f32, isOutput=False)
    bxc = nc.declare_dram_parameter("bxc", [P, NT, S], f32, isOutput=False)
    wt = nc.declare_dram_parameter("wt", [P, NT, CB, Cout], bf16, isOutput=False)
    kmat = nc.declare_dram_parameter("kmat", [P, 32], bf16, isOutput=False)
    perm = nc.declare_dram_parameter("perm", [P, 4, 4, P], bf16, isOutput=False)
    out = nc.declare_dram_parameter("out", [Cout, HW], bf16, isOutput=True)

    xtab_ap = bass.AP(xtab[:].tensor, 0, [[C, HW], [1, C]])

    with tile.TileContext(nc) as tc:
        with tc.tile_pool(name="persist", bufs=1) as pp:
            idx_sb = pp.tile([P, NSTRIPE, NT, SW], i16, name="idx_sb")
            oy = pp.tile([P, NT, S], f32, name="oy")
            ox = pp.tile([P, NT, S], f32, name="ox")
            by = pp.tile([P, NT, S], f32, name="by")
            bx = pp.tile([P, NT, S], f32, name="bx")
            # offsets first (they gate the serial phase-1 chain), then
            # stripe-0 indices so gathers start immediately after
            nc.sync.dma_start(out=oy[:], in_=offy[:])
            nc.sync.dma_start(out=ox[:], in_=offx[:])
            nc.sync.dma_start(out=by[:], in_=byc[:])
            nc.sync.dma_start(out=bx[:], in_=bxc[:])
            nc.sync.dma_start(out=idx_sb[:, 0], in_=idxp[0])
            perm_sb = pp.tile([P, 4, 4, P], bf16, name="perm_sb")
            nc.sync.dma_start(out=perm_sb[:], in_=perm[:])
            for st in range(1, NSTRIPE):
                nc.sync.dma_start(out=idx_sb[:, st], in_=idxp[st])
            wtb = pp.tile([P, NT, CB, Cout], bf16, name="wtb")
            nc.sync.dma_start(out=wtb[:], in_=wt[:])
            ksb = pp.tile([P, 32], bf16, name="ksb")
            nc.sync.dma_start(out=ksb[:], in_=kmat[:])
            # corner weights in corner-stacked layout: [p=32n+q, d, k, s]
            wcol = pp.tile([P, 4, NT, S], f32, name="wcol")

            with (
                tc.tile_pool(name="gather", bufs=12) as g_pool,
                tc.tile_pool(name="mtiles", bufs=8) as m_pool,
                tc.tile_pool(name="vout", bufs=3) as v_pool,
                tc.tile_pool(name="obuf", bufs=2) as o_pool,
                tc.tile_pool(name="psum_out", bufs=1, space="PSUM") as pso_pool,
                tc.tile_pool(name="psum_val", bufs=3, space="PSUM") as psv_pool,
            ):
                # ---- phase 1: bilinear corner weights (L128: px = 128s+p)
                with tc.tile_pool(name="ph1", bufs=1) as sp:
                    names = ["py", "px", "y0", "x0", "ly", "lx",
                             "my0", "my1", "mx0", "mx1",
                             "vy0", "vy1", "ux0", "ux1",
                             "w0", "w1", "w2", "w3", "sa", "sb"]
                    T = {nm: sp.tile([P, NT, S], f32, name=nm) for nm in names}
                    vi = sp.tile([P, NT, S], i32, name="vi")

                    def tt(o, a, b, op):
                        nc.vector.tensor_tensor(out=T[o][:], in0=T[a][:],
                                                in1=T[b][:], op=op)

                    def ts(o, a, s1, op0, s2=None, op1=None):
                        if s2 is None:
                            nc.vector.tensor_scalar(
                                out=T[o][:], in0=T[a][:], scalar1=float(s1),
                                scalar2=None, op0=op0,
                            )
                        else:
                            nc.vector.tensor_scalar(
                                out=T[o][:], in0=T[a][:], scalar1=float(s1),
                                scalar2=float(s2), op0=op0, op1=op1,
                            )

                    nc.vector.tensor_tensor(out=T["py"][:], in0=oy[:],
                                            in1=by[:], op=Alu.add)
                    nc.vector.tensor_tensor(out=T["px"][:], in0=ox[:],
                                            in1=bx[:], op=Alu.add)

                    def floor_(dst, src):
                        ts("sa", src, FADD, Alu.add)
                        nc.vector.tensor_copy(out=vi[:], in_=T["sa"][:])
                        nc.vector.tensor_copy(out=T["sb"][:], in_=vi[:])
                        ts(dst, "sb", FBIAS, Alu.subtract)

                    def rng_mask(dst, v, lo, hi):
                        # mask = (clamp(v, lo, hi) == v)
                        ts("sa", v, lo, Alu.max, hi, Alu.min)
                        tt(dst, "sa", v, Alu.is_equal)

                    floor_("y0", "py")
                    tt("ly", "py", "y0", Alu.subtract)
                    rng_mask("my0", "y0", 0.0, H - 1)
                    rng_mask("my1", "y0", -1.0, H - 2)
                    floor_("x0", "px")
                    tt("lx", "px", "x0", Alu.subtract)
                    rng_mask("mx0", "x0", 0.0, W - 1)
                    rng_mask("mx1", "x0", -1.0, W - 2)

                    ts("sa", "ly", -1.0, Alu.mult, 1.0, Alu.add)
                    tt("vy0", "sa", "my0", Alu.mult)
                    tt("vy1", "ly", "my1", Alu.mult)
                    ts("sb", "lx", -1.0, Alu.mult, 1.0, Alu.add)
                    tt("ux0", "sb", "mx0", Alu.mult)
                    tt("ux1", "lx", "mx1", Alu.mult)

                    tt("w0", "vy0", "ux0", Alu.mult)
                    tt("w1", "vy0", "ux1", Alu.mult)
                    tt("w2", "vy1", "ux0", Alu.mult)
                    tt("w3", "vy1", "ux1", Alu.mult)
                    w4 = [T["w0"], T["w1"], T["w2"], T["w3"]]

                    # shuffle into corner-stacked layout on the PE:
                    # wcol[32n+q, d, k, s] = w_n[32d+q, k, s] via 16 constant
                    # permutation matmuls (PSUM-accumulated over n per d)
                    w4b = sp.tile([P, 4, NT, S], bf16, name="w4b")
                    for n in range(4):
                        nc.vector.tensor_copy(out=w4b[:, n], in_=w4[n][:])
                    for d in range(4):
                        # reuse the out_ps bank slot (WAR-serialized, pre-loop)
                        wps = pso_pool.tile([P, 512], f32, space="PSUM",
                                            name="ops0")
                        for n in range(4):
                            nc.tensor.matmul(
                                out=wps[:, :NT * S],
                                lhsT=perm_sb[:, n, d, :],
                                rhs=w4b[:, n].rearrange("p k s -> p (k s)"),
                                start=(n == 0), stop=(n == 3),
                            )
                        nc.scalar.copy(
                            out=wcol[:, d].rearrange("p k s -> p (k s)"),
                            in_=wps[:, :NT * S])

                # ---- main loop
                for st in range(NSTRIPE):
                    out_ps = [
                        pso_pool.tile([P, SPX], f32, space="PSUM", name=f"ops{m}")
                        for m in range(MB)
                    ]
                    for k in range(NT):
                        gbuf = g_pool.tile([P, NG, C], bf16, name="gbuf")
                        # HW SWDGE limit: 1024 indices per gather instruction
                        nh = NIDX // 2
                        for h in range(2):
                            nc.gpsimd.dma_gather(
                                gbuf[:, h * (NG // 2):(h + 1) * (NG // 2), :],
                                xtab_ap,
                                idx_sb[:, st, k, h * (SW // 2):(h + 1) * (SW // 2)],
                                nh, nh, C,
                            )
                        mall = m_pool.tile([P, NG, 32], bf16, name="mall")
                        for sl in range(NSL):
                            for d in range(4):
                                g = 4 * sl + d
                                s_g = NSL * st + sl
                                nc.vector.tensor_scalar(
                                    out=mall[:, g, :], in0=ksb[:],
                                    scalar1=wcol[:, d, k, s_g:s_g + 1],
                                    scalar2=None, op0=Alu.mult,
                                )
                        val_ps = [
                            psv_pool.tile([P, SPX], f32, space="PSUM",
                                          name=f"vps{cb}")
                            for cb in range(CB)
                        ]
                        for sl in range(NSL):
                            for d in range(4):
                                g = 4 * sl + d
                                col = sl * P + 32 * d
                                for cb in range(CB):
                                    nc.tensor.matmul(
                                        out=val_ps[cb][:, col:col + 32],
                                        lhsT=gbuf[:, g, cb * P:(cb + 1) * P],
                                        rhs=mall[:, g, :],
                                        start=True, stop=True,
                                    )
                        vsb = v_pool.tile([P, CB, SPX], bf16, name="vsb")
                        nc.vector.tensor_copy(out=vsb[:, 0], in_=val_ps[0][:])
                        nc.scalar.copy(out=vsb[:, 1], in_=val_ps[1][:])
                        for mb in range(MB):
                            for cb in range(CB):
                                nc.tensor.matmul(
                                    out=out_ps[mb][:],
                                    lhsT=wtb[:, k, cb, mb * P:(mb + 1) * P],
                                    rhs=vsb[:, cb],
                                    start=(k == 0 and cb == 0),
                                    stop=(k == NT - 1 and cb == CB - 1),
                                )
                    for mb in range(MB):
                        ob = o_pool.tile([P, SPX], bf16, name="ob")
                        nc.scalar.copy(out=ob[:], in_=out_ps[mb][:])
                        nc.sync.dma_start(
                            out=out[mb * P:(mb + 1) * P, st * SPX:(st + 1) * SPX],
                            in_=ob[:],
                        )

    nc.compile()
    return nc


def host_prep(x_b, offset_b, weight, H, W, KH, KW, PAD):
    """Per-core input map from one batch slice (numpy, f32)."""
    C = x_b.shape[0]
    Cout = weight.shape[0]
    HW = H * W
    S = HW // P
    NT = KH * KW
    CB = C // P
    NSTRIPE = 8
    SPX = HW // NSTRIPE
    NIDX = 4 * SPX
    SW = NIDX // 16

    xt = np.ascontiguousarray(x_b.reshape(C, HW).T).astype(np_bf16)
    off = offset_b.reshape(NT, 2, HW)
    j = np.arange(HW)
    ks = np.arange(NT)
    byv = (j[None, :] // W - PAD + (ks // KW)[:, None]).astype(np.float32)  # [k, j]
    bxv = (j[None, :] % W - PAD + (ks % KW)[:, None]).astype(np.float32)

    def l128(a):  # [k, j] -> [p, k, s], j = 128*s + p
        return np.ascontiguousarray(a.reshape(NT, S, P).transpose(2, 0, 1)).astype(np.float32)

    # per-corner clamped gather rows, mirroring the device f32 floor trick
    py = off[:, 0].astype(np.float32) + byv
    px = off[:, 1].astype(np.float32) + bxv
    FADD = np.float32(4.0 * max(H, W) - 0.5)
    y0 = np.rint(py + FADD).astype(np.int64) - int(4.0 * max(H, W))
    x0 = np.rint(px + FADD).astype(np.int64) - int(4.0 * max(H, W))
    rq = np.zeros((4, NT, HW), np.int64)
    for n in range(4):
        yn = np.clip(y0 + (n >> 1), 0, H - 1)
        xn = np.clip(x0 + (n & 1), 0, W - 1)
        rq[n] = yn * W + xn

    # slot order per (stripe, k): i = 128*(4*sl+d) + 32*n + q,
    # pixel = stripe*SPX + 128*sl + 32*d + q
    i_arr = np.arange(NIDX)
    sl_i = i_arr // 512
    d_i = (i_arr // 128) % 4
    n_i = (i_arr % 128) // 32
    q_i = i_arr % 32
    pxl = 128 * sl_i + 32 * d_i + q_i      # [NIDX]
    idx = np.zeros((NSTRIPE, P, NT, SW), np.int16)
    for st in range(NSTRIPE):
        rows = rq[n_i, :, st * SPX + pxl].astype(np.int16)  # [NIDX, NT]
        wrap = rows.reshape(SW, 16, NT).transpose(1, 2, 0)  # [16, NT, SW]
        idx[st] = np.tile(wrap, (8, 1, 1))                  # [128, NT, SW]

    wr = weight.reshape(Cout, C, NT)
    wtv = wr.reshape(Cout, CB, P, NT).transpose(2, 3, 1, 0)  # [p, k, cb, o]
    kmat = (np.arange(P)[:, None] % 32 == np.arange(32)[None, :])
    # permutation constants: E[n,d][p,i] = 1 iff p == 32d+q and i == 32n+q
    E = np.zeros((4, 4, P, P), np.float32)
    q = np.arange(32)
    for n in range(4):
        for d in range(4):
            E[n, d, 32 * d + q, 32 * n + q] = 1.0
    permv = np.ascontiguousarray(E.transpose(2, 0, 1, 3))  # [p, n, d, i]

    return {
        "xtab": xt,
        "idxp": idx,
        "offy": l128(off[:, 0]), "offx": l128(off[:, 1]),
        "byc": l128(byv), "bxc": l128(bxv),
        "wt": np.ascontiguousarray(wtv).astype(np_bf16),
        "kmat": kmat.astype(np_bf16),
        "perm": permv.astype(np_bf16),
    }


_NC_CACHE = {}


def _get_nc(key, **kw):
    if key not in _NC_CACHE:
        _NC_CACHE[key] = build_dcn(**kw)
    return _NC_CACHE[key]


def kernel(x, offset, weight):
    x = np.asarray(x, dtype=np.float32)
    offset = np.asarray(offset, dtype=np.float32)
    weight = np.asarray(weight, dtype=np.float32)
    B, C, H, W = x.shape
    Cout = weight.shape[0]
    KH, KW = weight.shape[2], weight.shape[3]
    PAD = 1
    assert B == 8 and C % 128 == 0 and Cout % 128 == 0
    nc = _get_nc((C, Cout, H, W, KH, KW), C=C, Cout=Cout, H=H, W=W,
                 NT=KH * KW)
    in_maps = [host_prep(x[b], offset[b], weight, H, W, KH, KW, PAD)
               for b in range(B)]
    res = run_bass_kernel_spmd(nc, in_maps, list(range(B)))
    out = np.stack([res.results[b]["out"].astype(np.float32).reshape(Cout, H, W)
                    for b in range(B)])
    return out


# revision 15
# speedup vs baseline: 1.5581x; 1.0034x over previous
"""Self-contained Trainium2 (Bass/Tile) DeformConv2d kernel.

kernel(x, offset, weight) -> np.ndarray [B, Cout, H, W] float32.
Data-parallel over batch: one SPMD Bass program per NeuronCore (8 cores).

Per core (one image): the bf16 x^T table lives in DRAM and is gathered with
4 int16 indices per (tap, pixel) — one per bilinear corner — laid out so the
4 corners of 32 pixels fill the 128 gather partitions (partition = 32*n + q).
DVE computes the 4 bilinear corner weights on-chip (L128 layout), 16 small
SBUF->SBUF DMAs shuffle them into the corner-stacked layout, and one narrow
[128,32] TensorScalarPtr per (tap, 32-px group) builds M = diag(w) @ K.
A single PE matmul per (tap, group, c-half) then performs
scale+combine+transpose+sum in one shot: valT[c,j] = sum_p g[p,c] * M[p,j].
The per-tap GEMM accumulates out[o, px] over taps in PSUM.
"""
import sys
import numpy as np

for _p in ("/opt/trn_rl_repo",):
    if _p not in sys.path:
        sys.path.insert(0, _p)

import concourse.bass as bass
import concourse.mybir as mybir
import concourse.tile as tile
from concourse import bacc
from concourse.bass_utils import run_bass_kernel_spmd

try:
    from ml_dtypes import bfloat16 as np_bf16
except ImportError:  # jax ships ml_dtypes; fall back via jax.numpy
    import jax.numpy as _jnp
    np_bf16 = _jnp.bfloat16

f32 = mybir.dt.float32
bf16 = mybir.dt.bfloat16
i32 = mybir.dt.int32
i16 = mybir.dt.int16
Alu = mybir.AluOpType
P = 128


def build_dcn(C=256, Cout=256, H=64, W=64, NT=9, NSTRIPE=8):
    HW = H * W
    S = HW // P              # 32 (L128 free index; s = pixel // 128)
    CB = C // P              # 2
    MB = Cout // P           # 2
    SPX = HW // NSTRIPE      # 512 pixels per stripe
    NSL = SPX // P           # 4 s_locals per stripe
    NG = SPX // 32           # 16 groups of 32 px per stripe
    NIDX = 4 * SPX           # 2048 gather slots per (tap, stripe)
    SW = NIDX // 16          # 128 idx free slots (16-wrap)
    FBIAS = 4.0 * max(H, W)
    FADD = FBIAS - 0.5       # HW f32->int cast is round-nearest-even

    nc = bacc.Bacc("TRN2", target_bir_lowering=False, debug=False)

    xtab = nc.declare_dram_parameter("xtab", [HW, C], bf16, isOutput=False)
    idxp = nc.declare_dram_parameter("idxp", [NSTRIPE, P, NT, SW], i16,
                                     isOutput=False)
    offs = nc.declare_dram_parameter("offs", [P, 4, NT, S], f32, isOutput=False)
    wt = nc.declare_dram_parameter("wt", [P, NT, CB, Cout], bf16, isOutput=False)
    kmat = nc.declare_dram_parameter("kmat", [P, 32], bf16, isOutput=False)
    perm = nc.declare_dram_parameter("perm", [P, 4, 4, P], bf16, isOutput=False)
    out = nc.declare_dram_parameter("out", [Cout, HW], bf16, isOutput=True)

    xtab_ap = bass.AP(xtab[:].tensor, 0, [[C, HW], [1, C]])

    with tile.TileContext(nc) as tc:
        with tc.tile_pool(name="persist", bufs=1) as pp:
            idx_sb = pp.tile([P, NSTRIPE, NT, SW], i16, name="idx_sb")
            offs_sb = pp.tile([P, 4, NT, S], f32, name="offs_sb")
            # offsets first (they gate the serial phase-1 chain), then
            # stripe-0 indices so gathers start immediately after
            nc.sync.dma_start(out=offs_sb[:], in_=offs[:])
            nc.sync.dma_start(out=idx_sb[:, 0], in_=idxp[0])
            oy, ox, by, bx = (offs_sb[:, i] for i in range(4))
            perm_sb = pp.tile([P, 4, 4, P], bf16, name="perm_sb")
            nc.sync.dma_start(out=perm_sb[:], in_=perm[:])
            for st in range(1, NSTRIPE):
                nc.sync.dma_start(out=idx_sb[:, st], in_=idxp[st])
            wtb = pp.tile([P, NT, CB, Cout], bf16, name="wtb")
            nc.sync.dma_start(out=wtb[:], in_=wt[:])
            ksb = pp.tile([P, 32], bf16, name="ksb")
            nc.sync.dma_start(out=ksb[:], in_=kmat[:])
            # corner weights in corner-stacked layout: [p=32n+q, d, k, s]
            wcol = pp.tile([P, 4, NT, S], f32, name="wcol")

            with (
                tc.tile_pool(name="gather", bufs=12) as g_pool,
                tc.tile_pool(name="mtiles", bufs=8) as m_pool,
                tc.tile_pool(name="vout", bufs=3) as v_pool,
                tc.tile_pool(name="obuf", bufs=2) as o_pool,
                tc.tile_pool(name="psum_out", bufs=1, space="PSUM") as pso_pool,
                tc.tile_pool(name="psum_val", bufs=3, space="PSUM") as psv_pool,
            ):
                # ---- phase 1: bilinear corner weights (L128: px = 128s+p)
                with tc.tile_pool(name="ph1", bufs=1) as sp:
                    names = ["py", "px", "y0", "x0", "ly", "lx",
                             "my0", "my1", "mx0", "mx1",
                             "vy0", "vy1", "ux0", "ux1",
                             "w0", "w1", "w2", "w3", "sa", "sb"]
                    T = {nm: sp.tile([P, NT, S], f32, name=nm) for nm in names}
                    vi = sp.tile([P, NT, S], i32, name="vi")

                    def tt(o, a, b, op):
                        nc.vector.tensor_tensor(out=T[o][:], in0=T[a][:],
                                                in1=T[b][:], op=op)

                    def ts(o, a, s1, op0, s2=None, op1=None):
                        if s2 is None:
                            nc.vector.tensor_scalar(
                                out=T[o][:], in0=T[a][:], scalar1=float(s1),
                                scalar2=None, op0=op0,
                            )
                        else:
                            nc.vector.tensor_scalar(
                                out=T[o][:], in0=T[a][:], scalar1=float(s1),
                                scalar2=float(s2), op0=op0, op1=op1,
                            )

                    nc.vector.tensor_tensor(out=T["py"][:], in0=oy,
                                            in1=by, op=Alu.add)
                    nc.vector.tensor_tensor(out=T["px"][:], in0=ox,
                                            in1=bx, op=Alu.add)

                    def floor_(dst, src):
                        ts("sa", src, FADD, Alu.add)
                        nc.vector.tensor_copy(out=vi[:], in_=T["sa"][:])
                        nc.vector.tensor_copy(out=T["sb"][:], in_=vi[:])
                        ts(dst, "sb", FBIAS, Alu.subtract)

                    def rng_mask(dst, v, lo, hi):
                        # mask = (clamp(v, lo, hi) == v)
                        ts("sa", v, lo, Alu.max, hi, Alu.min)
                        tt(dst, "sa", v, Alu.is_equal)

                    floor_("y0", "py")
                    tt("ly", "py", "y0", Alu.subtract)
                    rng_mask("my0", "y0", 0.0, H - 1)
                    rng_mask("my1", "y0", -1.0, H - 2)
                    floor_("x0", "px")
                    tt("lx", "px", "x0", Alu.subtract)
                    rng_mask("mx0", "x0", 0.0, W - 1)
                    rng_mask("mx1", "x0", -1.0, W - 2)

                    ts("sa", "ly", -1.0, Alu.mult, 1.0, Alu.add)
                    tt("vy0", "sa", "my0", Alu.mult)
                    tt("vy1", "ly", "my1", Alu.mult)
                    ts("sb", "lx", -1.0, Alu.mult, 1.0, Alu.add)
                    tt("ux0", "sb", "mx0", Alu.mult)
                    tt("ux1", "lx", "mx1", Alu.mult)

                    tt("w0", "vy0", "ux0", Alu.mult)
                    tt("w1", "vy0", "ux1", Alu.mult)
                    tt("w2", "vy1", "ux0", Alu.mult)
                    tt("w3", "vy1", "ux1", Alu.mult)
                    w4 = [T["w0"], T["w1"], T["w2"], T["w3"]]

                    # shuffle into corner-stacked layout on the PE:
                    # wcol[32n+q, d, k, s] = w_n[32d+q, k, s] via 16 constant
                    # permutation matmuls (PSUM-accumulated over n per d)
                    w4b = sp.tile([P, 4, NT, S], bf16, name="w4b")
                    for n in range(4):
                        nc.vector.tensor_copy(out=w4b[:, n], in_=w4[n][:])
                    for d in range(4):
                        # reuse the out_ps bank slot (WAR-serialized, pre-loop)
                        wps = pso_pool.tile([P, 512], f32, space="PSUM",
                                            name="ops0")
                        for n in range(4):
                            nc.tensor.matmul(
                                out=wps[:, :NT * S],
                                lhsT=perm_sb[:, n, d, :],
                                rhs=w4b[:, n].rearrange("p k s -> p (k s)"),
                                start=(n == 0), stop=(n == 3),
                            )
                        nc.scalar.copy(
                            out=wcol[:, d].rearrange("p k s -> p (k s)"),
                            in_=wps[:, :NT * S])

                # ---- main loop
                for st in range(NSTRIPE):
                    out_ps = [
                        pso_pool.tile([P, SPX], f32, space="PSUM", name=f"ops{m}")
                        for m in range(MB)
                    ]
                    for k in range(NT):
                        # HW SWDGE limit: 1024 indices per gather instruction;
                        # separate half tiles so combines start per half
                        nh = NIDX // 2
                        ghalf = []
                        for h in range(2):
                            gb = g_pool.tile([P, NG // 2, C], bf16, name=f"gb{h}")
                            nc.gpsimd.dma_gather(
                                gb[:], xtab_ap,
                                idx_sb[:, st, k, h * (SW // 2):(h + 1) * (SW // 2)],
                                nh, nh, C,
                            )
                            ghalf.append(gb)
                        mall = m_pool.tile([P, NG, 32], bf16, name="mall")
                        for sl in range(NSL):
                            for d in range(4):
                                g = 4 * sl + d
                                s_g = NSL * st + sl
                                nc.vector.tensor_scalar(
                                    out=mall[:, g, :], in0=ksb[:],
                                    scalar1=wcol[:, d, k, s_g:s_g + 1],
                                    scalar2=None, op0=Alu.mult,
                                )
                        val_ps = [
                            psv_pool.tile([P, SPX], f32, space="PSUM",
                                          name=f"vps{cb}")
                            for cb in range(CB)
                        ]
                        for sl in range(NSL):
                            for d in range(4):
                                g = 4 * sl + d
                                col = sl * P + 32 * d
                                gb = ghalf[g // (NG // 2)]
                                gg = g % (NG // 2)
                                for cb in range(CB):
                                    nc.tensor.matmul(
                                        out=val_ps[cb][:, col:col + 32],
                                        lhsT=gb[:, gg, cb * P:(cb + 1) * P],
                                        rhs=mall[:, g, :],
                                        start=True, stop=True,
                                    )
                        vsb = v_pool.tile([P, CB, SPX], bf16, name="vsb")
                        nc.vector.tensor_copy(out=vsb[:, 0], in_=val_ps[0][:])
                        nc.scalar.copy(out=vsb[:, 1], in_=val_ps[1][:])
                        for mb in range(MB):
                            for cb in range(CB):
                                nc.tensor.matmul(
                                    out=out_ps[mb][:],
                                    lhsT=wtb[:, k, cb, mb * P:(mb + 1) * P],
                                    rhs=vsb[:, cb],
                                    start=(k == 0 and cb == 0),
                                    stop=(k == NT - 1 and cb == CB - 1),
                                )
                    for mb in range(MB):
                        ob = o_pool.tile([P, SPX], bf16, name="ob")
                        nc.scalar.copy(out=ob[:], in_=out_ps[mb][:])
                        nc.sync.dma_start(
                            out=out[mb * P:(mb + 1) * P, st * SPX:(st + 1) * SPX],
                            in_=ob[:],
                        )

    nc.compile()
    return nc


def host_prep(x_b, offset_b, weight, H, W, KH, KW, PAD):
    """Per-core input map from one batch slice (numpy, f32)."""
    C = x_b.shape[0]
    Cout = weight.shape[0]
    HW = H * W
    S = HW // P
    NT = KH * KW
    CB = C // P
    NSTRIPE = 8
    SPX = HW // NSTRIPE
    NIDX = 4 * SPX
    SW = NIDX // 16

    xt = np.ascontiguousarray(x_b.reshape(C, HW).T).astype(np_bf16)
    off = offset_b.reshape(NT, 2, HW)
    j = np.arange(HW)
    ks = np.arange(NT)
    byv = (j[None, :] // W - PAD + (ks // KW)[:, None]).astype(np.float32)  # [k, j]
    bxv = (j[None, :] % W - PAD + (ks % KW)[:, None]).astype(np.float32)

    def l128(a):  # [k, j] -> [p, k, s], j = 128*s + p
        return np.ascontiguousarray(a.reshape(NT, S, P).transpose(2, 0, 1)).astype(np.float32)

    # per-corner clamped gather rows, mirroring the device f32 floor trick
    py = off[:, 0].astype(np.float32) + byv
    px = off[:, 1].astype(np.float32) + bxv
    FADD = np.float32(4.0 * max(H, W) - 0.5)
    y0 = np.rint(py + FADD).astype(np.int64) - int(4.0 * max(H, W))
    x0 = np.rint(px + FADD).astype(np.int64) - int(4.0 * max(H, W))
    rq = np.zeros((4, NT, HW), np.int64)
    for n in range(4):
        yn = np.clip(y0 + (n >> 1), 0, H - 1)
        xn = np.clip(x0 + (n & 1), 0, W - 1)
        rq[n] = yn * W + xn

    # slot order per (stripe, k): i = 128*(4*sl+d) + 32*n + q,
    # pixel = stripe*SPX + 128*sl + 32*d + q
    i_arr = np.arange(NIDX)
    sl_i = i_arr // 512
    d_i = (i_arr // 128) % 4
    n_i = (i_arr % 128) // 32
    q_i = i_arr % 32
    pxl = 128 * sl_i + 32 * d_i + q_i      # [NIDX]
    idx = np.zeros((NSTRIPE, P, NT, SW), np.int16)
    for st in range(NSTRIPE):
        rows = rq[n_i, :, st * SPX + pxl].astype(np.int16)  # [NIDX, NT]
        wrap = rows.reshape(SW, 16, NT).transpose(1, 2, 0)  # [16, NT, SW]
        idx[st] = np.tile(wrap, (8, 1, 1))                  # [128, NT, SW]

    wr = weight.reshape(Cout, C, NT)
    wtv = wr.reshape(Cout, CB, P, NT).transpose(2, 3, 1, 0)  # [p, k, cb, o]
    kmat = (np.arange(P)[:, None] % 32 == np.arange(32)[None, :])
    # permutation constants: E[n,d][p,i] = 1 iff p == 32d+q and i == 32n+q
    E = np.zeros((4, 4, P, P), np.float32)
    q = np.arange(32)
    for n in range(4):
        for d in range(4):
            E[n, d, 32 * d + q, 32 * n + q] = 1.0
    permv = np.ascontiguousarray(E.transpose(2, 0, 1, 3))  # [p, n, d, i]

    return {
        "xtab": xt,
        "idxp": idx,
        "offs": np.stack([l128(off[:, 0]), l128(off[:, 1]),
                          l128(byv), l128(bxv)], axis=1),
        "wt": np.ascontiguousarray(wtv).astype(np_bf16),
        "kmat": kmat.astype(np_bf16),
        "perm": permv.astype(np_bf16),
    }


_NC_CACHE = {}


def _get_nc(key, **kw):
    if key not in _NC_CACHE:
        _NC_CACHE[key] = build_dcn(**kw)
    return _NC_CACHE[key]


def kernel(x, offset, weight):
    x = np.asarray(x, dtype=np.float32)
    offset = np.asarray(offset, dtype=np.float32)
    weight = np.asarray(weight, dtype=np.float32)
    B, C, H, W = x.shape
    Cout = weight.shape[0]
    KH, KW = weight.shape[2], weight.shape[3]
    PAD = 1
    assert B == 8 and C % 128 == 0 and Cout % 128 == 0
    nc = _get_nc((C, Cout, H, W, KH, KW), C=C, Cout=Cout, H=H, W=W,
                 NT=KH * KW)
    in_maps = [host_prep(x[b], offset[b], weight, H, W, KH, KW, PAD)
               for b in range(B)]
    res = run_bass_kernel_spmd(nc, in_maps, list(range(B)))
    out = np.stack([res.results[b]["out"].astype(np.float32).reshape(Cout, H, W)
                    for b in range(B)])
    return out
